# revision 27
# baseline (speedup 1.0000x reference)
"""Trainium2 Bass kernel for the attention-LSTM decoder step.

Strategy: data-parallel over batch B=64 across 8 NeuronCores (8 batches/core).
All compute in bf16 with fp32 PSUM accumulation. Host side pre-transposes /
pre-casts inputs (layout only; the one weight-fusion is G = W_gate @ events_mat,
a weights-only product).

Per-core pipeline:
  phase A: decT[g,b] = W_dec @ h0^T            (PE, bf16)
  phase B: per batch b:
    projT[g,s] = W_enc @ enc[b]^T  (+decT bias) -> tanh (ACT)
    scores[s]  = w_val^T tanh(projT)            (PE, M=1 accumulation)
    softmax    (DVE/ACT, [1,512] layout)
    attn broadcast to 128 partitions            (PE ones-outer-product)
    ctxT[h,b]  = sum_s attn[s]*encT[h,s]        (DVE fused mult+reduce)
  phase C: sim scores, LSTM gates = [ctx|sim|x|1|h] @ Wpad (PE), LSTM cell,
           LayerNorm, final projection.
"""

import numpy as np
import ml_dtypes

import concourse.bass as bass
import concourse.tile as tile
import concourse.mybir as mybir
from concourse.vector_clock import ScopedClock
from concourse.bass_utils import run_bass_kernel_spmd

BF = ml_dtypes.bfloat16
DT_BF = mybir.dt.bfloat16
DT_F32 = mybir.dt.float32
AF = mybir.ActivationFunctionType
ALU = mybir.AluOpType
AX = mybir.AxisListType

B, S, H, E = 64, 512, 1024, 10
NCORES = 8
BL = B // NCORES          # 8 local batches
GCH = H // 128            # 8 chunks of the hidden dim
KPAD = 17 * 128           # padded LSTM contraction (ctx 1024 | sim+x+1 pad 128 | h 1024)
G4 = 4 * H                # 4096 gate columns

_PATCHED = False


def _patch_tile_tail():
    """walrus on this image rejects >1 sem-wait on the tail Drain; split the
    waits across single-wait SP nops."""
    global _PATCHED
    if _PATCHED:
        return
    _PATCHED = True

    def _patched(self, tick_clock, wait_clock):
        nc = self.nc
        nop = nc.sync.nop(nofuse=True)
        wait_clock.add_sem_waits(
            nop.ins, ScopedClock({None: tick_clock.global_clock})
        )
        si = nop.ins.sync_info
        waits = list(si.on_wait) if si and si.on_wait else []
        if len(waits) > 1:
            si.on_wait = [waits[0]]
            for w in waits[1:]:
                n2 = nc.sync.nop(nofuse=True)
                n2.ins.sync_info = mybir.SyncInfo(on_wait=[w], on_update=[])
        nc.sync.drain()
        nc.all_engine_barrier()
        assert self.sems is not None
        popped = nc._tile_sem_poison_stack.pop()
        assert popped is self._sem_poison
        nc.clear_and_free_semaphores(list(self.sems.allocated().values()))
        nc.all_engine_barrier()

    tile.TileContext._drain_and_barrier = _patched


def _split_excess_waits(nc, limit=1):
    """walrus codegen on this image rejects >2 sem-waits per instruction.
    Move the excess onto injected same-engine nops placed right before the
    offending instruction (engines execute their stream in order, and a DMA
    trigger is dispatched by its issuing engine, so this is equivalent)."""
    k = 0
    for bb in nc.m.functions[0].blocks:
        insts = bb.instructions
        out = []
        for inst in insts:
            si = inst.sync_info
            waits = list(si.on_wait) if si and si.on_wait else []
            if len(waits) > limit:
                extra, keep = waits[:-limit], waits[-limit:]
                si.on_wait = keep
                for j in range(0, len(extra), limit):
                    k += 1
                    out.append(mybir.InstNoOp(
                        name=f"waitsplit-{k}",
                        engine=inst.engine,
                        bass_nofuse=True,
                        sync_info=mybir.SyncInfo(
                            on_wait=extra[j:j + limit], on_update=[]
                        ),
                    ))
            out.append(inst)
        bb.instructions = out


def _build():
    nc = bass.Bass("TRN2", debug=False)

    def din(name, shape, dt=DT_BF):
        return nc.declare_dram_parameter(name, list(shape), dt, isOutput=False)

    def dout(name, shape, dt=DT_F32):
        return nc.declare_dram_parameter(name, list(shape), dt, isOutput=True)

    enct = din("enct", [H, BL * S])
    wenct = din("wenct", [H, H])
    wdect = din("wdect", [H, H])
    wval = din("wval", [128, GCH])
    gt = din("gt", [128, GCH * E])
    wpad = din("wpad", [KPAD, G4])
    hts = din("hts", [128, GCH * BL])
    xo = din("xo", [2, BL])
    c0f = din("c0f", [BL, H], DT_F32)
    lngb = din("lngb", [BL, H], DT_F32)
    lnbb = din("lnbb", [BL, H], DT_F32)
    wfinb = din("wfinb", [BL, H], DT_F32)
    bfinb = din("bfinb", [BL, 1], DT_F32)

    o_fin = dout("fin", [BL, 1])
    o_h = dout("hnew", [BL, H])
    o_c = dout("cnew", [BL, H])
    o_attn = dout("attnw", [BL, S])
    o_sim = dout("simout", [BL, E])

    with tile.TileContext(nc) as tc:
        from contextlib import ExitStack

        ctx = ExitStack()
        with ctx:
            cpool = ctx.enter_context(tc.tile_pool(name="consts", bufs=1))
            wenc_p = ctx.enter_context(tc.tile_pool(name="wenc", bufs=GCH))
            enc_p = ctx.enter_context(tc.tile_pool(name="enc", bufs=12))
            tanh_p = ctx.enter_context(tc.tile_pool(name="tanh", bufs=4))
            junk_p = ctx.enter_context(tc.tile_pool(name="junk", bufs=2))
            sm_p = ctx.enter_context(tc.tile_pool(name="sm", bufs=2))
            misc_p = ctx.enter_context(tc.tile_pool(name="misc", bufs=1))

            decT_sb = misc_p.tile([128, GCH * BL], DT_F32, tag="decT")
            ctxT_f = misc_p.tile([128, GCH * BL], DT_F32, tag="ctxTf")
            ctxT_b = misc_p.tile([128, GCH * BL], DT_BF, tag="ctxTb")

            enc_t = {}

            def load_bg(bg):
                for ci in range(GCH):
                    t = enc_p.tile([128, 2 * S], DT_BF, tag="enc",
                                   name=f"enc{bg}_{ci}")
                    nc.sync.dma_start(
                        t[:],
                        enct.ap()[
                            ci * 128:(ci + 1) * 128, bg * 2 * S:(bg + 1) * 2 * S
                        ],
                    )
                    enc_t[(bg, ci)] = t

            def enc_rhs(b, ci):
                bg, half = divmod(b, 2)
                return enc_t[(bg, ci)][:, half * S:(half + 1) * S]

            # ---- SP-ring DMA order: phase-A weights first, then the first
            # two batch-groups of enc, then the stage-1 weights + misc. ----
            hts_sb = cpool.tile([128, GCH * BL], DT_BF, tag="hts")
            nc.sync.dma_start(hts_sb[:], hts.ap()[:])
            with tc.tile_pool(name="wdec", bufs=GCH) as wdec_p:
                wdec_sb = []
                for ci in range(GCH):
                    t = wdec_p.tile([128, H], DT_BF, tag="wdec")
                    nc.sync.dma_start(t[:], wdect.ap()[ci * 128:(ci + 1) * 128, :])
                    wdec_sb.append(t)
                load_bg(0)
                wenc_sb = []
                for ci in range(GCH):
                    t = wenc_p.tile([128, H], DT_BF, tag="wenc")
                    nc.sync.dma_start(t[:], wenct.ap()[ci * 128:(ci + 1) * 128, :])
                    wenc_sb.append(t)
                wval_sb = cpool.tile([128, GCH], DT_BF, tag="wval")
                nc.sync.dma_start(wval_sb[:], wval.ap()[:])
                gt_sb = cpool.tile([128, GCH * E], DT_BF, tag="gt")
                nc.sync.dma_start(gt_sb[:], gt.ap()[:])
                c0_sb = cpool.tile([BL, H], DT_F32, tag="c0")
                nc.sync.dma_start(c0_sb[:], c0f.ap()[:])
                lng_sb = cpool.tile([BL, H], DT_F32, tag="lng")
                nc.sync.dma_start(lng_sb[:], lngb.ap()[:])
                lnb_sb = cpool.tile([BL, H], DT_F32, tag="lnb")
                nc.sync.dma_start(lnb_sb[:], lnbb.ap()[:])
                wfin_sb = cpool.tile([BL, H], DT_F32, tag="wfin")
                nc.sync.dma_start(wfin_sb[:], wfinb.ap()[:])
                bfin_sb = cpool.tile([BL, 1], DT_F32, tag="bfin")
                nc.sync.dma_start(bfin_sb[:], bfinb.ap()[:])
                ones_sb = cpool.tile([1, 128], DT_BF, tag="ones")
                nc.vector.memset(ones_sb[:], 1.0)
                # preload every ACT function table we use so no table-swap
                # stalls land in the serial tail
                warm = cpool.tile([1, 2], DT_F32, tag="warm")
                nc.vector.memset(warm[:], 0.5)
                for fn in (AF.Tanh, AF.Exp, AF.Sigmoid, AF.Square, AF.Sqrt):
                    nc.scalar.activation(warm[:], warm[:], fn)
                load_bg(1)

                # ---- phase A: decT[g, b]; ci-outer so each wdec tile is
                # consumed as soon as its DMA lands ----
                with tc.tile_pool(name="psd", bufs=1, space="PSUM") as psd:
                    dpss = [
                        psd.tile([128, BL], DT_F32, tag=f"dec{gi}",
                                 name=f"dec{gi}")
                        for gi in range(GCH)
                    ]
                    for ci in range(GCH):
                        for gi in range(GCH):
                            nc.tensor.matmul(
                                dpss[gi][:],
                                lhsT=wdec_sb[ci][:, gi * 128:(gi + 1) * 128],
                                rhs=hts_sb[:, ci * BL:(ci + 1) * BL],
                                start=(ci == 0),
                                stop=(ci == GCH - 1),
                                skip_group_check=True,
                            )
                    for gi in range(GCH):
                        nc.scalar.copy(
                            decT_sb[:, gi * BL:(gi + 1) * BL], dpss[gi][:]
                        )

            # ---- LSTM weight prefetch on the ACT HW-DGE ring (its own FIFO,
            # so it streams during phase B instead of queueing behind enc).
            # Emission order MUST match the gates k-loop consumption order so
            # the pool cycles without deadlock. ----
            wpad_p = ctx.enter_context(tc.tile_pool(name="wpad", bufs=22))
            GATE_K_ORDER = [*range(9, 17), 8, *range(8)]
            WPAD_PAIRS = [(ci, q) for ci in GATE_K_ORDER for q in range(2)]
            wpad_t = {}

            def load_wpad(pairs):
                for ci, q in pairs:
                    t = wpad_p.tile([128, 2048], DT_BF, tag="wpad",
                                    name=f"wp{ci}_{q}")
                    nc.sync.dma_start(
                        t[:],
                        wpad.ap()[ci * 128:(ci + 1) * 128, q * 2048:(q + 1) * 2048],
                    )
                    wpad_t[(ci, q)] = t

            load_wpad(WPAD_PAIRS[:18])

            def wpad_rhs(ci, n):
                return wpad_t[(ci, n // 4)][:, (n % 4) * 512:(n % 4 + 1) * 512]

            # ---- phase B ----
            with (
                tc.tile_pool(name="psp", bufs=3, space="PSUM") as psp,
                tc.tile_pool(name="pss", bufs=2, space="PSUM") as pss,
                tc.tile_pool(name="psa", bufs=2, space="PSUM") as psa,
            ):
                score_ps = {}
                pend_score = None
                pend_post = None

                def emit_score(b, gi):
                    def go():
                        tt = tanh_tiles[(b, gi)]
                        nc.tensor.matmul(
                            score_ps[b][:],
                            lhsT=wval_sb[:, gi:gi + 1],
                            rhs=tt[:],
                            start=(gi == 0),
                            stop=(gi == GCH - 1),
                            skip_group_check=True,
                        )
                    return go

                def emit_post(b):
                    # softmax + attn output + broadcast + context accumulation
                    def go():
                        sp = score_ps.pop(b)
                        negmx = sm_p.tile([1, 1], DT_F32, tag="negmx")
                        nc.vector.tensor_reduce(
                            negmx[:], sp[:], axis=AX.X, op=ALU.max, negate=True
                        )
                        exp_sb = sm_p.tile([1, S], DT_F32, tag="exp")
                        ssum = sm_p.tile([1, 1], DT_F32, tag="ssum")
                        nc.scalar.activation(
                            exp_sb[:], sp[:], AF.Exp, bias=negmx[:], accum_out=ssum[:]
                        )
                        rinv = sm_p.tile([1, 1], DT_F32, tag="rinv")
                        nc.vector.reciprocal(rinv[:], ssum[:])
                        attn_f = sm_p.tile([1, S], DT_F32, tag="attnf")
                        nc.vector.tensor_scalar(
                            attn_f[:], exp_sb[:], rinv[:], None, ALU.mult
                        )
                        nc.scalar.dma_start(o_attn.ap()[b:b + 1, :], attn_f[:])
                        attn_b = sm_p.tile([1, S], DT_BF, tag="attnb")
                        nc.vector.tensor_copy(attn_b[:], attn_f[:])
                        aps = psa.tile([128, S], DT_F32, tag="attnb")
                        nc.tensor.matmul(
                            aps[:], lhsT=ones_sb[:], rhs=attn_b[:],
                            start=True, stop=True,
                        )
                        for ci in range(GCH):
                            jt = junk_p.tile([128, S], DT_BF, tag="junk")
                            nc.vector.scalar_tensor_tensor(
                                out=jt[:],
                                in0=enc_rhs(b, ci),
                                scalar=1.0,
                                op0=ALU.bypass,
                                in1=aps[:],
                                op1=ALU.mult,
                                accum_out=ctxT_f[:, ci * BL + b:ci * BL + b + 1],
                            )
                    return go

                tanh_tiles = {}
                for b in range(BL):
                    if b % 2 == 0 and 2 <= b <= 4:
                        load_bg(b // 2 + 1)
                    if b == 6:
                        load_wpad(WPAD_PAIRS[18:])
                    score_ps[b] = pss.tile(
                        [1, S], DT_F32, tag="score", name=f"score{b}"
                    )
                    for gi in range(GCH):
                        proj = psp.tile([128, S], DT_F32, tag="proj")
                        for ci in range(GCH):
                            nc.tensor.matmul(
                                proj[:],
                                lhsT=wenc_sb[ci][:, gi * 128:(gi + 1) * 128],
                                rhs=enc_rhs(b, ci),
                                start=(ci == 0),
                                stop=(ci == GCH - 1),
                            )
                        tt = tanh_p.tile([128, S], DT_BF, tag="tanh")
                        col = gi * BL + b
                        nc.scalar.activation(
                            tt[:], proj[:], AF.Tanh, bias=decT_sb[:, col:col + 1]
                        )
                        tanh_tiles[(b, gi)] = tt
                        if pend_score is not None:
                            pend_score()
                            pend_score = None
                        if pend_post is not None:
                            pend_post()
                            pend_post = None
                        pend_score = emit_score(b, gi)
                    # end gi loop: defer score(b,7) + post(b) past proj(b+1,g0)
                    pend_post_prev = pend_post
                    assert pend_post_prev is None
                    pend_post = emit_post(b)
                # epilogue
                if pend_score is not None:
                    pend_score()
                if pend_post is not None:
                    pend_post()

            # ---- phase C ----
            nc.vector.tensor_copy(ctxT_b[:], ctxT_f[:])

            with tc.tile_pool(name="lstm", bufs=1) as lp:
              with tc.tile_pool(name="psim", bufs=1, space="PSUM") as psim:
                simT_ps = psim.tile([E, BL], DT_F32, tag="simT")
                for ci in range(GCH):
                    nc.tensor.matmul(
                        simT_ps[:],
                        lhsT=gt_sb[:, ci * E:(ci + 1) * E],
                        rhs=ctxT_b[:, ci * BL:(ci + 1) * BL],
                        start=(ci == 0),
                        stop=(ci == GCH - 1),
                    )
                sim_ps = psim.tile([BL, E], DT_F32, tag="sim")
                for ci in range(GCH):
                    nc.tensor.matmul(
                        sim_ps[:],
                        lhsT=ctxT_b[:, ci * BL:(ci + 1) * BL],
                        rhs=gt_sb[:, ci * E:(ci + 1) * E],
                        start=(ci == 0),
                        stop=(ci == GCH - 1),
                    )
                sim_f = lp.tile([BL, E], DT_F32, tag="simf")
                nc.scalar.copy(sim_f[:], sim_ps[:])
                nc.scalar.dma_start(o_sim.ap()[:], sim_f[:])
                simT_b = lp.tile([E, BL], DT_BF, tag="simTb")
                nc.vector.tensor_copy(simT_b[:], simT_ps[:])

                xo_sb = lp.tile([128, BL], DT_BF, tag="xosb")
                nc.scalar.dma_start(xo_sb[96:98, :], xo.ap()[:])
                mix = lp.tile([128, BL], DT_BF, tag="mix")
                nc.vector.memset(mix[:], 0.0)
                nc.vector.tensor_copy(mix[0:E, :], simT_b[:])
                nc.vector.tensor_copy(mix[96:98, :], xo_sb[96:98, :])

              def gate_lhs(ci):
                    if ci < 8:
                        return ctxT_b[:, ci * BL:(ci + 1) * BL]
                    if ci == 8:
                        return mix[:]
                    return hts_sb[:, (ci - 9) * BL:(ci - 8) * BL]

              with tc.tile_pool(name="psg", bufs=1, space="PSUM") as psg:
                # gates: k-major, 2 PSUM banks x 4 PE column-groups.
                # n -> (bank n%2, col-group n//2); the 4 col-groups run
                # concurrently in the PE array (tile_position packing).
                gbank = [
                    psg.tile([128, S], DT_F32, tag=f"gb{m}", name=f"gb{m}")
                    for m in range(2)
                ]
                gps = [gbank[n % 2][32 * (n // 2):32 * (n // 2) + BL, :]
                       for n in range(8)]
                for j, ci in enumerate(GATE_K_ORDER):
                    for n in range(8):
                        nc.tensor.matmul(
                            gps[n],
                            lhsT=gate_lhs(ci),
                            rhs=wpad_rhs(ci, n),
                            start=(j == 0),
                            stop=(j == 16),
                            skip_group_check=True,
                            tile_position=(0, 32 * (n // 2)),
                        )
                sigi = lp.tile([BL, H], DT_F32, tag="t0")
                sigf = lp.tile([BL, H], DT_F32, tag="t1")
                tang = lp.tile([BL, H], DT_F32, tag="t2")
                sigo = lp.tile([BL, H], DT_F32, tag="t3")
                gact = [
                    (sigi, 0, AF.Sigmoid), (sigi, 1, AF.Sigmoid),
                    (sigf, 0, AF.Sigmoid), (sigf, 1, AF.Sigmoid),
                    (tang, 0, AF.Tanh), (tang, 1, AF.Tanh),
                    (sigo, 0, AF.Sigmoid), (sigo, 1, AF.Sigmoid),
                ]
                for n in range(8):
                    dst, half, fn = gact[n]
                    nc.scalar.activation(
                        dst[:, half * S:(half + 1) * S], gps[n], fn
                    )

                # cell + LayerNorm in two 512-column halves so ACT and DVE
                # pipeline instead of serializing on [8,1024] ops
                cnew = lp.tile([BL, H], DT_F32, tag="cnew")
                hnew = lp.tile([BL, H], DT_F32, tag="hnew")
                hsums, dhs = [], []
                for q in range(2):
                    sl = slice(q * S, (q + 1) * S)
                    ta = lp.tile([BL, S], DT_F32, tag=f"ta{q}", name=f"ta{q}")
                    nc.vector.tensor_mul(ta[:], sigf[:, sl], c0_sb[:, sl])
                    tb = lp.tile([BL, S], DT_F32, tag=f"tb{q}", name=f"tb{q}")
                    nc.vector.tensor_mul(tb[:], sigi[:, sl], tang[:, sl])
                    nc.vector.tensor_add(cnew[:, sl], ta[:], tb[:])
                    nc.scalar.dma_start(o_c.ap()[:, sl], cnew[:, sl])
                    th = lp.tile([BL, S], DT_F32, tag=f"th{q}", name=f"th{q}")
                    nc.scalar.activation(th[:], cnew[:, sl], AF.Tanh)
                    hs = lp.tile([BL, 1], DT_F32, tag=f"hs{q}", name=f"hs{q}")
                    nc.vector.scalar_tensor_tensor(
                        out=hnew[:, sl], in0=sigo[:, sl], scalar=1.0,
                        op0=ALU.mult, in1=th[:], op1=ALU.mult, accum_out=hs[:],
                    )
                    nc.scalar.dma_start(o_h.ap()[:, sl], hnew[:, sl])
                    hsums.append(hs)

                hsum = lp.tile([BL, 1], DT_F32, tag="hsum")
                nc.vector.tensor_add(hsum[:], hsums[0][:], hsums[1][:])
                mu = lp.tile([BL, 1], DT_F32, tag="mu")
                nc.scalar.mul(mu[:], hsum[:], 1.0 / H)
                sqs = []
                for q in range(2):
                    sl = slice(q * S, (q + 1) * S)
                    dq = lp.tile([BL, S], DT_F32, tag=f"d{q}", name=f"d{q}")
                    nc.vector.tensor_scalar(
                        dq[:], hnew[:, sl], mu[:], None, ALU.subtract
                    )
                    dsq = lp.tile([BL, S], DT_F32, tag=f"ta{q}", name=f"dsq{q}")
                    sq = lp.tile([BL, 1], DT_F32, tag=f"sq{q}", name=f"sq{q}")
                    nc.scalar.activation(dsq[:], dq[:], AF.Square, accum_out=sq[:])
                    dhs.append(dq)
                    sqs.append(sq)
                sqsum = lp.tile([BL, 1], DT_F32, tag="sqsum")
                nc.vector.tensor_add(sqsum[:], sqs[0][:], sqs[1][:])
                eps = lp.tile([BL, 1], DT_F32, tag="eps")
                nc.vector.memset(eps[:], 1e-5)
                sigma = lp.tile([BL, 1], DT_F32, tag="sigma")
                nc.scalar.activation(
                    sigma[:], sqsum[:], AF.Sqrt, scale=1.0 / H, bias=eps[:]
                )
                rstd = lp.tile([BL, 1], DT_F32, tag="rstd")
                nc.vector.reciprocal(rstd[:], sigma[:])
                fins = []
                for q in range(2):
                    sl = slice(q * S, (q + 1) * S)
                    y1 = lp.tile([BL, S], DT_F32, tag=f"tb{q}", name=f"y1_{q}")
                    nc.vector.scalar_tensor_tensor(
                        out=y1[:], in0=dhs[q][:], scalar=rstd[:], op0=ALU.mult,
                        in1=lng_sb[:, sl], op1=ALU.mult,
                    )
                    y2 = lp.tile([BL, S], DT_F32, tag=f"th{q}", name=f"y2_{q}")
                    nc.vector.tensor_add(y2[:], y1[:], lnb_sb[:, sl])
                    yj = lp.tile([BL, S], DT_F32, tag=f"ta{q}", name=f"yj{q}")
                    fc = lp.tile([BL, 1], DT_F32, tag=f"fc{q}", name=f"fc{q}")
                    nc.vector.scalar_tensor_tensor(
                        out=yj[:], in0=y2[:], scalar=1.0, op0=ALU.bypass,
                        in1=wfin_sb[:, sl], op1=ALU.mult, accum_out=fc[:],
                    )
                    fins.append(fc)
                fin_c = lp.tile([BL, 1], DT_F32, tag="fin")
                nc.vector.tensor_add(fin_c[:], fins[0][:], fins[1][:])
                fin_o = lp.tile([BL, 1], DT_F32, tag="fino")
                nc.vector.tensor_add(fin_o[:], fin_c[:], bfin_sb[:])
                nc.scalar.dma_start(o_fin.ap()[:], fin_o[:])

    return nc


def _prep(inputs):
    """Host-side shard + layout prep. Returns per-core input dicts."""
    f32 = np.float32
    x = np.asarray(inputs["x"], f32)
    h0 = np.asarray(inputs["h0"], f32)[0]
    c0 = np.asarray(inputs["c0"], f32)[0]
    enc = np.asarray(inputs["encoder_output"], f32)
    W_enc = np.asarray(inputs["W_enc"], f32)
    W_dec = np.asarray(inputs["W_dec"], f32)
    w_val = np.asarray(inputs["w_val"], f32)
    W_gate = np.asarray(inputs["W_gate"], f32)
    events = np.asarray(inputs["events_mat"], f32)
    W_ih = np.asarray(inputs["W_ih"], f32)
    W_hh = np.asarray(inputs["W_hh"], f32)
    b_ih = np.asarray(inputs["b_ih"], f32)
    b_hh = np.asarray(inputs["b_hh"], f32)
    ln_g = np.asarray(inputs["ln_g"], f32)
    ln_b = np.asarray(inputs["ln_b"], f32)
    W_fin = np.asarray(inputs["W_fin"], f32)
    b_fin = np.asarray(inputs["b_fin"], f32)

    G = W_gate @ events                      # [E, H] weights-only fusion
    wenct = np.ascontiguousarray(W_enc.T).astype(BF)
    wdect = np.ascontiguousarray(W_dec.T).astype(BF)
    wvalm = np.ascontiguousarray(w_val.reshape(GCH, 128).T).astype(BF)
    gtm = np.ascontiguousarray(
        G.T.reshape(GCH, 128, E).transpose(1, 0, 2).reshape(128, GCH * E)
    ).astype(BF)
    wpad = np.zeros((KPAD, G4), f32)
    W_ihT = W_ih.T
    wpad[0:H] = W_ihT[0:H]
    wpad[H:H + E] = W_ihT[H:H + E]        # sim rows -> mix partitions 0..9
    wpad[H + 96] = W_ihT[H + E]           # x row    -> mix partition 96
    wpad[H + 97] = b_ih + b_hh            # bias row -> mix partition 97 (ones)
    wpad[9 * 128:] = W_hh.T
    wpad = wpad.astype(BF)

    maps = []
    for k in range(NCORES):
        sl = slice(k * BL, (k + 1) * BL)
        enct = np.ascontiguousarray(
            enc[sl].transpose(2, 0, 1).reshape(H, BL * S)
        ).astype(BF)
        hloc = h0[sl]                          # [BL, H]
        hts = np.ascontiguousarray(
            hloc.T.reshape(GCH, 128, BL).transpose(1, 0, 2).reshape(128, GCH * BL)
        ).astype(BF)
        maps.append({
            "enct": enct,
            "wenct": wenct,
            "wdect": wdect,
            "wval": wvalm,
            "gt": gtm,
            "wpad": wpad,
            "hts": hts,
            "xo": np.ascontiguousarray(
                np.stack([x[sl, 0], np.ones(BL, np.float32)])
            ).astype(BF),
            "c0f": np.ascontiguousarray(c0[sl]),
            "lngb": np.ascontiguousarray(np.broadcast_to(ln_g, (BL, H))),
            "lnbb": np.ascontiguousarray(np.broadcast_to(ln_b, (BL, H))),
            "wfinb": np.ascontiguousarray(np.broadcast_to(W_fin[0], (BL, H))),
            "bfinb": np.full((BL, 1), b_fin[0], f32),
        })
    return maps


def _run(inputs, trace=False, **kw):
    _patch_tile_tail()
    nc = _build()
    _split_excess_waits(nc)
    in_maps = _prep(inputs)
    res = run_bass_kernel_spmd(nc, in_maps, list(range(NCORES)), trace=trace, **kw)
    r = res.results
    fin = np.concatenate([r[k]["fin"] for k in range(NCORES)], axis=0)
    h_new = np.concatenate([r[k]["hnew"] for k in range(NCORES)], axis=0)[None]
    c_new = np.concatenate([r[k]["cnew"] for k in range(NCORES)], axis=0)[None]
    attn = np.concatenate([r[k]["attnw"] for k in range(NCORES)], axis=0)
    sim = np.concatenate([r[k]["simout"] for k in range(NCORES)], axis=0)
    return (fin, (h_new, c_new), attn, sim), res


def kernel(**inputs):
    out, _ = _run(inputs)
    return out


# revision 41
# speedup vs baseline: 1.4047x; 1.4047x over previous
"""Trainium2 Bass kernel for the attention-LSTM decoder step.

Strategy: data-parallel over batch B=64 across 8 NeuronCores (8 batches/core).
All compute in bf16 with fp32 PSUM accumulation. Host side pre-transposes /
pre-casts inputs (layout only; the one weight-fusion is G = W_gate @ events_mat,
a weights-only product).

Per-core pipeline:
  phase A: decT[g,b] = W_dec @ h0^T            (PE, bf16)
  phase B: per batch b:
    projT[g,s] = W_enc @ enc[b]^T  (+decT bias) -> tanh (ACT)
    scores[s]  = w_val^T tanh(projT)            (PE, M=1 accumulation)
    softmax    (DVE/ACT, [1,512] layout)
    attn broadcast to 128 partitions            (PE ones-outer-product)
    ctxT[h,b]  = sum_s attn[s]*encT[h,s]        (DVE fused mult+reduce)
  phase C: sim scores, LSTM gates = [ctx|sim|x|1|h] @ Wpad (PE), LSTM cell,
           LayerNorm, final projection.
"""

import numpy as np
import ml_dtypes

import concourse.bass as bass
import concourse.tile as tile
import concourse.mybir as mybir
from concourse.vector_clock import ScopedClock
from concourse.bass_utils import run_bass_kernel_spmd

BF = ml_dtypes.bfloat16
DT_BF = mybir.dt.bfloat16
DT_F32 = mybir.dt.float32
AF = mybir.ActivationFunctionType
ALU = mybir.AluOpType
AX = mybir.AxisListType

B, S, H, E = 64, 512, 1024, 10
NCORES = 8
BL = B // NCORES          # 8 local batches
GCH = H // 128            # 8 chunks of the hidden dim
KPAD = 17 * 128           # padded LSTM contraction (ctx 1024 | sim+x+1 pad 128 | h 1024)
G4 = 4 * H                # 4096 gate columns

_PATCHED = False


def _patch_tile_tail():
    """walrus on this image rejects >1 sem-wait on the tail Drain; split the
    waits across single-wait SP nops."""
    global _PATCHED
    if _PATCHED:
        return
    _PATCHED = True

    def _patched(self, tick_clock, wait_clock):
        nc = self.nc
        nop = nc.sync.nop(nofuse=True)
        wait_clock.add_sem_waits(
            nop.ins, ScopedClock({None: tick_clock.global_clock})
        )
        si = nop.ins.sync_info
        waits = list(si.on_wait) if si and si.on_wait else []
        if len(waits) > 1:
            si.on_wait = [waits[0]]
            for w in waits[1:]:
                n2 = nc.sync.nop(nofuse=True)
                n2.ins.sync_info = mybir.SyncInfo(on_wait=[w], on_update=[])
        nc.sync.drain()
        nc.all_engine_barrier()
        assert self.sems is not None
        popped = nc._tile_sem_poison_stack.pop()
        assert popped is self._sem_poison
        nc.clear_and_free_semaphores(list(self.sems.allocated().values()))
        # no trailing barrier: nothing follows the clears in this execution,
        # and NEFF completion already waits for every engine stream to end

    tile.TileContext._drain_and_barrier = _patched


def _split_excess_waits(nc, limit=1):
    """walrus codegen on this image rejects >2 sem-waits per instruction.
    Move the excess onto injected same-engine nops placed right before the
    offending instruction (engines execute their stream in order, and a DMA
    trigger is dispatched by its issuing engine, so this is equivalent)."""
    k = 0
    for bb in nc.m.functions[0].blocks:
        insts = bb.instructions
        out = []
        for inst in insts:
            si = inst.sync_info
            waits = list(si.on_wait) if si and si.on_wait else []
            if len(waits) > limit:
                extra, keep = waits[:-limit], waits[-limit:]
                si.on_wait = keep
                for j in range(0, len(extra), limit):
                    k += 1
                    out.append(mybir.InstNoOp(
                        name=f"waitsplit-{k}",
                        engine=inst.engine,
                        bass_nofuse=True,
                        sync_info=mybir.SyncInfo(
                            on_wait=extra[j:j + limit], on_update=[]
                        ),
                    ))
            out.append(inst)
        bb.instructions = out


def _build():
    nc = bass.Bass("TRN2", debug=False)

    def din(name, shape, dt=DT_BF):
        return nc.declare_dram_parameter(name, list(shape), dt, isOutput=False)

    def dout(name, shape, dt=DT_F32):
        return nc.declare_dram_parameter(name, list(shape), dt, isOutput=True)

    enct = din("enct", [H, BL * S])
    wenct = din("wenct", [128, GCH * H])
    wdect = din("wdect", [128, GCH * H])
    wval = din("wval", [128, GCH])
    gt = din("gt", [128, GCH * E])
    wpad = din("wpad", [KPAD, G4])
    hts = din("hts", [128, GCH * BL])
    xo = din("xo", [2, BL])
    c0f = din("c0f", [BL, H], DT_F32)
    lngb = din("lngb", [BL, H], DT_F32)
    lnbb = din("lnbb", [BL, H], DT_F32)
    wfinb = din("wfinb", [BL, H], DT_F32)
    bfinb = din("bfinb", [BL, 1], DT_F32)

    o_fin = dout("fin", [BL, 1])
    o_h = dout("hnew", [BL, H])
    o_c = dout("cnew", [BL, H])
    o_attn = dout("attnw", [BL, S])
    o_sim = dout("simout", [BL, E])

    with tile.TileContext(nc) as tc:
        from contextlib import ExitStack

        ctx = ExitStack()
        with ctx:
            cpool = ctx.enter_context(tc.tile_pool(name="consts", bufs=1))
            wenc_p = ctx.enter_context(tc.tile_pool(name="wenc", bufs=1))
            misc_p = ctx.enter_context(tc.tile_pool(name="misc", bufs=1))
            wpad_p = ctx.enter_context(tc.tile_pool(name="wpad", bufs=24))
            enc_p = ctx.enter_context(tc.tile_pool(name="enc", bufs=16))
            tanh_p = ctx.enter_context(tc.tile_pool(name="tanh", bufs=6))
            junk_p = ctx.enter_context(tc.tile_pool(name="junk", bufs=2))
            sm_p = ctx.enter_context(tc.tile_pool(name="sm", bufs=2))

            decT_sb = misc_p.tile([128, GCH * BL], DT_F32, tag="decT")
            mix = misc_p.tile([128, BL], DT_BF, tag="mix")
            ctxT_f = misc_p.tile([128, GCH * BL], DT_F32, tag="ctxTf")
            ctxT_b = misc_p.tile([128, GCH * BL], DT_BF, tag="ctxTb")

            enc_t = {}

            def load_bg(bg):
                for ci in range(GCH):
                    t = enc_p.tile([128, 2 * S], DT_BF, tag="enc",
                                   name=f"enc{bg}_{ci}")
                    nc.sync.dma_start(
                        t[:],
                        enct.ap()[
                            ci * 128:(ci + 1) * 128, bg * 2 * S:(bg + 1) * 2 * S
                        ],
                    )
                    enc_t[(bg, ci)] = t

            def enc_rhs(b, ci):
                bg, half = divmod(b, 2)
                return enc_t[(bg, ci)][:, half * S:(half + 1) * S]

            # ---- SP-ring DMA order: phase-A weights first, then the first
            # two batch-groups of enc, then the stage-1 weights + misc. ----
            hts_sb = cpool.tile([128, GCH * BL], DT_BF, tag="hts")
            nc.sync.dma_start(hts_sb[:], hts.ap()[:])
            with tc.tile_pool(name="wdec", bufs=1) as wdec_p:
                wdec_sb = wdec_p.tile([128, GCH * H], DT_BF, tag="wdec")
                nc.sync.dma_start(
                    wdec_sb[:, 0:2 * H], wdect.ap()[:, 0:2 * H]
                )
                nc.sync.dma_start(
                    wdec_sb[:, 2 * H:4 * H], wdect.ap()[:, 2 * H:4 * H]
                )
                nc.sync.dma_start(
                    wdec_sb[:, 4 * H:GCH * H], wdect.ap()[:, 4 * H:GCH * H]
                )
                # stage-1 weights in two halves (g0-3 first) + first batches
                wenc_sb = wenc_p.tile([128, GCH * H], DT_BF, tag="wenc")
                nc.sync.dma_start(
                    wenc_sb[:, 0:4 * H], wenct.ap()[:, 0:4 * H]
                )
                load_bg(0)
                nc.sync.dma_start(
                    wenc_sb[:, 4 * H:GCH * H], wenct.ap()[:, 4 * H:GCH * H]
                )
                load_bg(1)
                wval_sb = cpool.tile([128, GCH], DT_BF, tag="wval")
                nc.sync.dma_start(wval_sb[:], wval.ap()[:])
                gt_sb = cpool.tile([128, GCH * E], DT_BF, tag="gt")
                nc.sync.dma_start(gt_sb[:], gt.ap()[:])
                c0_sb = cpool.tile([BL, H], DT_F32, tag="c0")
                nc.sync.dma_start(c0_sb[:], c0f.ap()[:])
                lng_sb = cpool.tile([BL, H], DT_F32, tag="lng")
                nc.sync.dma_start(lng_sb[:], lngb.ap()[:])
                lnb_sb = cpool.tile([BL, H], DT_F32, tag="lnb")
                nc.sync.dma_start(lnb_sb[:], lnbb.ap()[:])
                wfin_sb = cpool.tile([BL, H], DT_F32, tag="wfin")
                nc.sync.dma_start(wfin_sb[:], wfinb.ap()[:])
                bfin_sb = cpool.tile([BL, 1], DT_F32, tag="bfin")
                nc.sync.dma_start(bfin_sb[:], bfinb.ap()[:])
                ones_sb = cpool.tile([1, 128], DT_BF, tag="ones")
                nc.vector.memset(ones_sb[:], 1.0)
                xo_sb = cpool.tile([128, BL], DT_BF, tag="xosb")
                nc.sync.dma_start(xo_sb[96:98, :], xo.ap()[:])
                # preload every ACT function table we use so no table-swap
                # stalls land in the serial tail
                warm = cpool.tile([1, 2], DT_F32, tag="warm")
                nc.vector.memset(warm[:], 0.5)
                for fn in (AF.Tanh, AF.Exp, AF.Sigmoid, AF.Square, AF.Sqrt):
                    nc.scalar.activation(warm[:], warm[:], fn)

                # ---- phase A: decT[g, b]; ci-outer so each wdec tile is
                # consumed as soon as its DMA lands ----
                with tc.tile_pool(name="psd", bufs=1, space="PSUM") as psd:
                    dpss = [
                        psd.tile([128, BL], DT_F32, tag=f"dec{gi}",
                                 name=f"dec{gi}")
                        for gi in range(GCH)
                    ]
                    for ci in range(GCH):
                        for gi in range(GCH):
                            nc.tensor.matmul(
                                dpss[gi][:],
                                lhsT=wdec_sb[:, ci * H + gi * 128:
                                             ci * H + (gi + 1) * 128],
                                rhs=hts_sb[:, ci * BL:(ci + 1) * BL],
                                start=(ci == 0),
                                stop=(ci == GCH - 1),
                                skip_group_check=True,
                            )
                    for gi in range(GCH):
                        nc.scalar.copy(
                            decT_sb[:, gi * BL:(gi + 1) * BL], dpss[gi][:]
                        )

            # ---- LSTM weight prefetch on the ACT HW-DGE ring (its own FIFO,
            # so it streams during phase B instead of queueing behind enc).
            # Emission order MUST match the gates k-loop consumption order so
            # the pool cycles without deadlock. ----
            GATE_K_ORDER = [*range(9, 17), *range(8), 8]
            WPAD_PAIRS = [(ci, q) for ci in GATE_K_ORDER for q in range(2)]
            wpad_t = {}

            def load_wpad(pairs):
                for ci, q in pairs:
                    t = wpad_p.tile([128, 2048], DT_BF, tag="wpad",
                                    name=f"wp{ci}_{q}")
                    nc.sync.dma_start(
                        t[:],
                        wpad.ap()[ci * 128:(ci + 1) * 128, q * 2048:(q + 1) * 2048],
                    )
                    wpad_t[(ci, q)] = t

            load_wpad(WPAD_PAIRS[:18])

            def wpad_rhs(ci, n):
                return wpad_t[(ci, n // 4)][:, (n % 4) * 512:(n % 4 + 1) * 512]

            # ---- phase B ----
            psg = ctx.enter_context(tc.tile_pool(name="psg", bufs=1,
                                                 space="PSUM"))
            gbank = [
                psg.tile([128, S], DT_F32, tag=f"gb{m}", name=f"gb{m}")
                for m in range(2)
            ]

            def gate_lhs(ci):
                if ci < 8:
                    return ctxT_b[:, ci * BL:(ci + 1) * BL]
                if ci == 8:
                    return mix[:]
                return hts_sb[:, (ci - 9) * BL:(ci - 8) * BL]

            # gates: k-major, 2 PSUM banks x 4 PE column-groups.
            gps = [gbank[n % 2][32 * (n // 2):32 * (n // 2) + BL, :]
                   for n in range(8)]

            def gate_k(j, ci):
                for n in range(8):
                    nc.tensor.matmul(
                        gps[n],
                        lhsT=gate_lhs(ci),
                        rhs=wpad_rhs(ci, n),
                        start=(j == 0),
                        stop=(j == 16),
                        skip_group_check=True,
                        tile_position=(0, 32 * (n // 2)),
                    )
            with (
                tc.tile_pool(name="psp", bufs=2, space="PSUM") as psp,
                tc.tile_pool(name="pss", bufs=2, space="PSUM") as pss,
                tc.tile_pool(name="psa", bufs=2, space="PSUM") as psa,
            ):
                score_ps = {}
                pend_score = None
                pend_post = None

                def emit_score(b, gi):
                    def go():
                        tt = tanh_tiles[(b, gi)]
                        nc.tensor.matmul(
                            score_ps[b][:],
                            lhsT=wval_sb[:, gi:gi + 1],
                            rhs=tt[:],
                            start=(gi == 0),
                            stop=(gi == GCH - 1),
                            skip_group_check=True,
                        )
                    return go

                def emit_post(b):
                    # softmax + attn output + broadcast + context accumulation
                    def go():
                        sp = score_ps.pop(b)
                        negmx = sm_p.tile([1, 1], DT_F32, tag="negmx")
                        nc.vector.tensor_reduce(
                            negmx[:], sp[:], axis=AX.X, op=ALU.max, negate=True
                        )
                        exp_sb = sm_p.tile([1, S], DT_F32, tag="exp")
                        ssum = sm_p.tile([1, 1], DT_F32, tag="ssum")
                        nc.scalar.activation(
                            exp_sb[:], sp[:], AF.Exp, bias=negmx[:], accum_out=ssum[:]
                        )
                        rinv = sm_p.tile([1, 1], DT_F32, tag="rinv")
                        nc.vector.reciprocal(rinv[:], ssum[:])
                        attn_f = sm_p.tile([1, S], DT_F32, tag="attnf")
                        nc.vector.tensor_scalar(
                            attn_f[:], exp_sb[:], rinv[:], None, ALU.mult
                        )
                        nc.scalar.dma_start(o_attn.ap()[b:b + 1, :], attn_f[:])
                        attn_b = sm_p.tile([1, S], DT_BF, tag="attnb")
                        nc.vector.tensor_copy(attn_b[:], attn_f[:])
                        aps = psa.tile([128, S], DT_F32, tag="attnb")
                        nc.tensor.matmul(
                            aps[:], lhsT=ones_sb[:], rhs=attn_b[:],
                            start=True, stop=True,
                        )
                        for ci in range(GCH):
                            jt = junk_p.tile([128, S], DT_BF, tag="junk")
                            nc.vector.scalar_tensor_tensor(
                                out=jt[:],
                                in0=enc_rhs(b, ci),
                                scalar=1.0,
                                op0=ALU.bypass,
                                in1=aps[:],
                                op1=ALU.mult,
                                accum_out=ctxT_f[:, ci * BL + b:ci * BL + b + 1],
                            )
                    return go

                tanh_tiles = {}
                for b in range(BL):
                    if b % 2 == 0 and 2 <= b <= 4:
                        load_bg(b // 2 + 1)
                    if b == 6:
                        load_wpad(WPAD_PAIRS[18:])
                    score_ps[b] = pss.tile(
                        [1, S], DT_F32, tag="score", name=f"score{b}"
                    )
                    for gi in range(GCH):
                        proj = psp.tile([128, S], DT_F32, tag="proj")
                        for ci in range(GCH):
                            nc.tensor.matmul(
                                proj[:],
                                lhsT=wenc_sb[:, gi * H + ci * 128:
                                             gi * H + (ci + 1) * 128],
                                rhs=enc_rhs(b, ci),
                                start=(ci == 0),
                                stop=(ci == GCH - 1),
                            )
                        tt = tanh_p.tile([128, S], DT_BF, tag="tanh")
                        col = gi * BL + b
                        nc.scalar.activation(
                            tt[:], proj[:], AF.Tanh, bias=decT_sb[:, col:col + 1]
                        )
                        tanh_tiles[(b, gi)] = tt
                        if pend_score is not None:
                            pend_score()
                            pend_score = None
                        if pend_post is not None:
                            pend_post()
                            pend_post = None
                        pend_score = emit_score(b, gi)
                    # end gi loop: defer score(b,7) + post(b) past proj(b+1,g0)
                    pend_post_prev = pend_post
                    assert pend_post_prev is None
                    pend_post = emit_post(b)
                # epilogue: the h-dependent gates chunks fill the PE while
                # the last batch's softmax/broadcast/context run on ACT/DVE
                if pend_score is not None:
                    pend_score()
                for j in range(8):
                    gate_k(j, GATE_K_ORDER[j])
                if pend_post is not None:
                    pend_post()

            # ---- phase C ----
            nc.vector.tensor_copy(ctxT_b[:], ctxT_f[:])

            with tc.tile_pool(name="lstm", bufs=1) as lp:
              # keep the Sqrt ACT table resident for the LayerNorm tail
              # (phase-B Exp/Tanh traffic evicts it)
              nc.scalar.activation(warm[:], warm[:], AF.Sqrt)
              with tc.tile_pool(name="psim", bufs=1, space="PSUM") as psim:
                simT_ps = psim.tile([E, BL], DT_F32, tag="simT")
                for ci in range(GCH):
                    nc.tensor.matmul(
                        simT_ps[:],
                        lhsT=gt_sb[:, ci * E:(ci + 1) * E],
                        rhs=ctxT_b[:, ci * BL:(ci + 1) * BL],
                        start=(ci == 0),
                        stop=(ci == GCH - 1),
                    )
                sim_ps = psim.tile([BL, E], DT_F32, tag="sim")
                for ci in range(GCH):
                    nc.tensor.matmul(
                        sim_ps[:],
                        lhsT=ctxT_b[:, ci * BL:(ci + 1) * BL],
                        rhs=gt_sb[:, ci * E:(ci + 1) * E],
                        start=(ci == 0),
                        stop=(ci == GCH - 1),
                    )
                sim_f = lp.tile([BL, E], DT_F32, tag="simf")
                nc.scalar.copy(sim_f[:], sim_ps[:])
                nc.scalar.dma_start(o_sim.ap()[:], sim_f[:])
                simT_b = lp.tile([E, BL], DT_BF, tag="simTb")
                nc.vector.tensor_copy(simT_b[:], simT_ps[:])

                nc.vector.memset(mix[:], 0.0)
                nc.vector.tensor_copy(mix[0:E, :], simT_b[:])
                nc.vector.tensor_copy(mix[96:98, :], xo_sb[96:98, :])
                for j in range(8, 17):
                    gate_k(j, GATE_K_ORDER[j])

              if True:
                sigi = lp.tile([BL, H], DT_F32, tag="t0")
                sigf = lp.tile([BL, H], DT_F32, tag="t1")
                tang = lp.tile([BL, H], DT_F32, tag="t2")
                sigo = lp.tile([BL, H], DT_F32, tag="t3")
                gact = [
                    (sigi, 0, AF.Sigmoid), (sigi, 1, AF.Sigmoid),
                    (sigf, 0, AF.Sigmoid), (sigf, 1, AF.Sigmoid),
                    (tang, 0, AF.Tanh), (tang, 1, AF.Tanh),
                    (sigo, 0, AF.Sigmoid), (sigo, 1, AF.Sigmoid),
                ]
                for n in (2, 3, 0, 1, 4, 5, 6, 7):
                    dst, half, fn = gact[n]
                    nc.scalar.activation(
                        dst[:, half * S:(half + 1) * S], gps[n], fn
                    )

                ta = lp.tile([BL, H], DT_F32, tag="t4")
                nc.vector.tensor_mul(ta[:], sigf[:], c0_sb[:])
                tb = lp.tile([BL, H], DT_F32, tag="t5")
                nc.vector.tensor_mul(tb[:], sigi[:], tang[:])
                cnew = lp.tile([BL, H], DT_F32, tag="t6")
                nc.vector.tensor_add(cnew[:], ta[:], tb[:])
                nc.scalar.dma_start(o_c.ap()[:], cnew[:])
                tanhc = lp.tile([BL, H], DT_F32, tag="t4")
                nc.scalar.activation(tanhc[:], cnew[:], AF.Tanh)
                hnew = lp.tile([BL, H], DT_F32, tag="t5")
                hsum = lp.tile([BL, 1], DT_F32, tag="hsum")
                nc.vector.scalar_tensor_tensor(
                    out=hnew[:], in0=sigo[:], scalar=1.0, op0=ALU.mult,
                    in1=tanhc[:], op1=ALU.mult, accum_out=hsum[:],
                )
                nc.scalar.dma_start(o_h.ap()[:], hnew[:])

                mu = lp.tile([BL, 1], DT_F32, tag="mu")
                nc.scalar.mul(mu[:], hsum[:], 1.0 / H)
                d = lp.tile([BL, H], DT_F32, tag="t0")
                nc.vector.tensor_scalar(d[:], hnew[:], mu[:], None, ALU.subtract)
                dsq = lp.tile([BL, H], DT_F32, tag="t1")
                sqsum = lp.tile([BL, 1], DT_F32, tag="sqsum")
                nc.scalar.activation(dsq[:], d[:], AF.Square, accum_out=sqsum[:])
                eps = lp.tile([BL, 1], DT_F32, tag="eps")
                nc.vector.memset(eps[:], 1e-5)
                sigma = lp.tile([BL, 1], DT_F32, tag="sigma")
                nc.scalar.activation(
                    sigma[:], sqsum[:], AF.Sqrt, scale=1.0 / H, bias=eps[:]
                )
                rstd = lp.tile([BL, 1], DT_F32, tag="rstd")
                nc.vector.reciprocal(rstd[:], sigma[:])
                y1 = lp.tile([BL, H], DT_F32, tag="t3")
                nc.vector.scalar_tensor_tensor(
                    out=y1[:], in0=d[:], scalar=rstd[:], op0=ALU.mult,
                    in1=lng_sb[:], op1=ALU.mult,
                )
                y2 = lp.tile([BL, H], DT_F32, tag="t6")
                nc.vector.tensor_add(y2[:], y1[:], lnb_sb[:])
                yj = lp.tile([BL, H], DT_F32, tag="t4")
                fin_c = lp.tile([BL, 1], DT_F32, tag="fin")
                nc.vector.scalar_tensor_tensor(
                    out=yj[:], in0=y2[:], scalar=1.0, op0=ALU.bypass,
                    in1=wfin_sb[:], op1=ALU.mult, accum_out=fin_c[:],
                )
                fin_o = lp.tile([BL, 1], DT_F32, tag="fino")
                nc.vector.tensor_add(fin_o[:], fin_c[:], bfin_sb[:])
                nc.scalar.dma_start(o_fin.ap()[:], fin_o[:])

    return nc.declare_dram_parameter(name, list(shape), dt, isOutput=False)

    def dout(name, shape, dt=DT_F32):
        return nc.declare_dram_parameter(name, list(shape), dt, isOutput=True)

    enct = din("enct", [H, BL * S])
    wenct = din("wenct", [128, GCH * H])
    wdect = din("wdect", [128, GCH * H])
    wval = din("wval", [128, GCH])
    gt = din("gt", [128, GCH * E])
    wpad = din("wpad", [KPAD, G4])
    hts = din("hts", [128, GCH * BL])
    xo = din("xo", [2, BL])
    c0f = din("c0f", [BL, H], DT_F32)
    lngb = din("lngb", [BL, H], DT_F32)
    lnbb = din("lnbb", [BL, H], DT_F32)
    wfinb = din("wfinb", [BL, H], DT_F32)
    bfinb = din("bfinb", [BL, 1], DT_F32)

    o_fin = dout("fin", [BL, 1])
    o_h = dout("hnew", [BL, H])
    o_c = dout("cnew", [BL, H])
    o_attn = dout("attnw", [BL, S])
    o_sim = dout("simout", [BL, E])

    with tile.TileContext(nc) as tc:
        from contextlib import ExitStack

        ctx = ExitStack()
        with ctx:
            cpool = ctx.enter_context(tc.tile_pool(name="consts", bufs=1))
            wenc_p = ctx.enter_context(tc.tile_pool(name="wenc", bufs=1))
            misc_p = ctx.enter_context(tc.tile_pool(name="misc", bufs=1))
            wpad_p = ctx.enter_context(tc.tile_pool(name="wpad", bufs=24))
            enc_p = ctx.enter_context(tc.tile_pool(name="enc", bufs=16))
            tanh_p = ctx.enter_context(tc.tile_pool(name="tanh", bufs=6))
            junk_p = ctx.enter_context(tc.tile_pool(name="junk", bufs=2))
            sm_p = ctx.enter_context(tc.tile_pool(name="sm", bufs=2))

            decT_sb = misc_p.tile([128, GCH * BL], DT_F32, tag="decT")
            mix = misc_p.tile([128, BL], DT_BF, tag="mix")
            ctxT_f = misc_p.tile([128, GCH * BL], DT_F32, tag="ctxTf")
            ctxT_b = misc_p.tile([128, GCH * BL], DT_BF, tag="ctxTb")

            enc_t = {}

            def load_bg(bg):
                for ci in range(GCH):
                    t = enc_p.tile([128, 2 * S], DT_BF, tag="enc",
                                   name=f"enc{bg}_{ci}")
                    nc.sync.dma_start(
                        t[:],
                        enct.ap()[
                            ci * 128:(ci + 1) * 128, bg * 2 * S:(bg + 1) * 2 * S
                        ],
                    )
                    enc_t[(bg, ci)] = t

            def enc_rhs(b, ci):
                bg, half = divmod(b, 2)
                return enc_t[(bg, ci)][:, half * S:(half + 1) * S]

            # ---- SP-ring DMA order: phase-A weights first, then the first
            # two batch-groups of enc, then the stage-1 weights + misc. ----
            hts_sb = cpool.tile([128, GCH * BL], DT_BF, tag="hts")
            nc.sync.dma_start(hts_sb[:], hts.ap()[:])
            with tc.tile_pool(name="wdec", bufs=1) as wdec_p:
                wdec_sb = wdec_p.tile([128, GCH * H], DT_BF, tag="wdec")
                nc.sync.dma_start(
                    wdec_sb[:, 0:2 * H], wdect.ap()[:, 0:2 * H]
                )
                nc.sync.dma_start(
                    wdec_sb[:, 2 * H:4 * H], wdect.ap()[:, 2 * H:4 * H]
                )
                nc.sync.dma_start(
                    wdec_sb[:, 4 * H:GCH * H], wdect.ap()[:, 4 * H:GCH * H]
                )
                # stage-1 weights in two halves (g0-3 first) + first batches
                wenc_sb = wenc_p.tile([128, GCH * H], DT_BF, tag="wenc")
                nc.sync.dma_start(
                    wenc_sb[:, 0:4 * H], wenct.ap()[:, 0:4 * H]
                )
                load_bg(0)
                nc.sync.dma_start(
                    wenc_sb[:, 4 * H:GCH * H], wenct.ap()[:, 4 * H:GCH * H]
                )
                load_bg(1)
                wval_sb = cpool.tile([128, GCH], DT_BF, tag="wval")
                nc.sync.dma_start(wval_sb[:], wval.ap()[:])
                gt_sb = cpool.tile([128, GCH * E], DT_BF, tag="gt")
                nc.sync.dma_start(gt_sb[:], gt.ap()[:])
                c0_sb = cpool.tile([BL, H], DT_F32, tag="c0")
                nc.sync.dma_start(c0_sb[:], c0f.ap()[:])
                lng_sb = cpool.tile([BL, H], DT_F32, tag="lng")
                nc.sync.dma_start(lng_sb[:], lngb.ap()[:])
                lnb_sb = cpool.tile([BL, H], DT_F32, tag="lnb")
                nc.sync.dma_start(lnb_sb[:], lnbb.ap()[:])
                wfin_sb = cpool.tile([BL, H], DT_F32, tag="wfin")
                nc.sync.dma_start(wfin_sb[:], wfinb.ap()[:])
                bfin_sb = cpool.tile([BL, 1], DT_F32, tag="bfin")
                nc.sync.dma_start(bfin_sb[:], bfinb.ap()[:])
                ones_sb = cpool.tile([1, 128], DT_BF, tag="ones")
                nc.vector.memset(ones_sb[:], 1.0)
                xo_sb = cpool.tile([128, BL], DT_BF, tag="xosb")
                nc.sync.dma_start(xo_sb[96:98, :], xo.ap()[:])
                # preload every ACT function table we use so no table-swap
                # stalls land in the serial tail
                warm = cpool.tile([1, 2], DT_F32, tag="warm")
                nc.vector.memset(warm[:], 0.5)
                for fn in (AF.Tanh, AF.Exp, AF.Sigmoid, AF.Square, AF.Sqrt):
                    nc.scalar.activation(warm[:], warm[:], fn)

                # ---- phase A: decT[g, b]; ci-outer so each wdec tile is
                # consumed as soon as its DMA lands ----
                with tc.tile_pool(name="psd", bufs=1, space="PSUM") as psd:
                    dpss = [
                        psd.tile([128, BL], DT_F32, tag=f"dec{gi}",
                                 name=f"dec{gi}")
                        for gi in range(GCH)
                    ]
                    for ci in range(GCH):
                        for gi in range(GCH):
                            nc.tensor.matmul(
                                dpss[gi][:],
                                lhsT=wdec_sb[:, ci * H + gi * 128:
                                             ci * H + (gi + 1) * 128],
                                rhs=hts_sb[:, ci * BL:(ci + 1) * BL],
                                start=(ci == 0),
                                stop=(ci == GCH - 1),
                                skip_group_check=True,
                            )
                    for gi in range(GCH):
                        nc.scalar.copy(
                            decT_sb[:, gi * BL:(gi + 1) * BL], dpss[gi][:]
                        )

            # ---- LSTM weight prefetch on the ACT HW-DGE ring (its own FIFO,
            # so it streams during phase B instead of queueing behind enc).
            # Emission order MUST match the gates k-loop consumption order so
            # the pool cycles without deadlock. ----
            GATE_K_ORDER = [*range(9, 17), *range(8), 8]
            WPAD_PAIRS = [(ci, q) for ci in GATE_K_ORDER for q in range(2)]
            wpad_t = {}

            def load_wpad(pairs):
                for ci, q in pairs:
                    t = wpad_p.tile([128, 2048], DT_BF, tag="wpad",
                                    name=f"wp{ci}_{q}")
                    nc.sync.dma_start(
                        t[:],
                        wpad.ap()[ci * 128:(ci + 1) * 128, q * 2048:(q + 1) * 2048],
                    )
                    wpad_t[(ci, q)] = t

            load_wpad(WPAD_PAIRS[:18])

            def wpad_rhs(ci, n):
                return wpad_t[(ci, n // 4)][:, (n % 4) * 512:(n % 4 + 1) * 512]

            # ---- phase B ----
            psg = ctx.enter_context(tc.tile_pool(name="psg", bufs=1,
                                                 space="PSUM"))
            gbank = [
                psg.tile([128, S], DT_F32, tag=f"gb{m}", name=f"gb{m}")
                for m in range(2)
            ]

            def gate_lhs(ci):
                if ci < 8:
                    return ctxT_b[:, ci * BL:(ci + 1) * BL]
                if ci == 8:
                    return mix[:]
                return hts_sb[:, (ci - 9) * BL:(ci - 8) * BL]

            # gates: k-major, 2 PSUM banks x 4 PE column-groups.
            gps = [gbank[n % 2][32 * (n // 2):32 * (n // 2) + BL, :]
                   for n in range(8)]

            def gate_k(j, ci):
                for n in range(8):
                    nc.tensor.matmul(
                        gps[n],
                        lhsT=gate_lhs(ci),
                        rhs=wpad_rhs(ci, n),
                        start=(j == 0),
                        stop=(j == 16),
                        skip_group_check=True,
                        tile_position=(0, 32 * (n // 2)),
                    )
            with (
                tc.tile_pool(name="psp", bufs=2, space="PSUM") as psp,
                tc.tile_pool(name="pss", bufs=2, space="PSUM") as pss,
                tc.tile_pool(name="psa", bufs=2, space="PSUM") as psa,
            ):
                score_ps = {}
                pend_score = None
                pend_post = None

                def emit_score(b, gi):
                    def go():
                        tt = tanh_tiles[(b, gi)]
                        nc.tensor.matmul(
                            score_ps[b][:],
                            lhsT=wval_sb[:, gi:gi + 1],
                            rhs=tt[:],
                            start=(gi == 0),
                            stop=(gi == GCH - 1),
                            skip_group_check=True,
                        )
                    return go

                def emit_post(b):
                    # softmax + attn output + broadcast + context accumulation
                    def go():
                        sp = score_ps.pop(b)
                        negmx = sm_p.tile([1, 1], DT_F32, tag="negmx")
                        nc.vector.tensor_reduce(
                            negmx[:], sp[:], axis=AX.X, op=ALU.max, negate=True
                        )
                        exp_sb = sm_p.tile([1, S], DT_F32, tag="exp")
                        ssum = sm_p.tile([1, 1], DT_F32, tag="ssum")
                        nc.scalar.activation(
                            exp_sb[:], sp[:], AF.Exp, bias=negmx[:], accum_out=ssum[:]
                        )
                        rinv = sm_p.tile([1, 1], DT_F32, tag="rinv")
                        nc.vector.reciprocal(rinv[:], ssum[:])
                        attn_f = sm_p.tile([1, S], DT_F32, tag="attnf")
                        nc.vector.tensor_scalar(
                            attn_f[:], exp_sb[:], rinv[:], None, ALU.mult
                        )
                        nc.scalar.dma_start(o_attn.ap()[b:b + 1, :], attn_f[:])
                        attn_b = sm_p.tile([1, S], DT_BF, tag="attnb")
                        nc.vector.tensor_copy(attn_b[:], attn_f[:])
                        aps = psa.tile([128, S], DT_F32, tag="attnb")
                        nc.tensor.matmul(
                            aps[:], lhsT=ones_sb[:], rhs=attn_b[:],
                            start=True, stop=True,
                        )
                        for ci in range(GCH):
                            jt = junk_p.tile([128, S], DT_BF, tag="junk")
                            nc.vector.scalar_tensor_tensor(
                                out=jt[:],
                                in0=enc_rhs(b, ci),
                                scalar=1.0,
                                op0=ALU.bypass,
                                in1=aps[:],
                                op1=ALU.mult,
                                accum_out=ctxT_f[:, ci * BL + b:ci * BL + b + 1],
                            )
                    return go

                tanh_tiles = {}
                for b in range(BL):
                    if b % 2 == 0 and 2 <= b <= 4:
                        load_bg(b // 2 + 1)
                    if b == 6:
                        load_wpad(WPAD_PAIRS[18:])
                    score_ps[b] = pss.tile(
                        [1, S], DT_F32, tag="score", name=f"score{b}"
                    )
                    for gi in range(GCH):
                        proj = psp.tile([128, S], DT_F32, tag="proj")
                        for ci in range(GCH):
                            nc.tensor.matmul(
                                proj[:],
                                lhsT=wenc_sb[:, gi * H + ci * 128:
                                             gi * H + (ci + 1) * 128],
                                rhs=enc_rhs(b, ci),
                                start=(ci == 0),
                                stop=(ci == GCH - 1),
                            )
                        tt = tanh_p.tile([128, S], DT_BF, tag="tanh")
                        col = gi * BL + b
                        nc.scalar.activation(
                            tt[:], proj[:], AF.Tanh, bias=decT_sb[:, col:col + 1]
                        )
                        tanh_tiles[(b, gi)] = tt
                        if pend_score is not None:
                            pend_score()
                            pend_score = None
                        if pend_post is not None:
                            pend_post()
                            pend_post = None
                        pend_score = emit_score(b, gi)
                    # end gi loop: defer score(b,7) + post(b) past proj(b+1,g0)
                    pend_post_prev = pend_post
                    assert pend_post_prev is None
                    pend_post = emit_post(b)
                # epilogue: the h-dependent gates chunks fill the PE while
                # the last batch's softmax/broadcast/context run on ACT/DVE
                if pend_score is not None:
                    pend_score()
                for j in range(8):
                    gate_k(j, GATE_K_ORDER[j])
                if pend_post is not None:
                    pend_post()

            # ---- phase C ----
            nc.vector.tensor_copy(ctxT_b[:], ctxT_f[:])

            with tc.tile_pool(name="lstm", bufs=1) as lp:
              # keep the Sqrt ACT table resident for the LayerNorm tail
              # (phase-B Exp/Tanh traffic evicts it)
              nc.scalar.activation(warm[:], warm[:], AF.Sqrt)
              with tc.tile_pool(name="psim", bufs=1, space="PSUM") as psim:
                simT_ps = psim.tile([E, BL], DT_F32, tag="simT")
                for ci in range(GCH):
                    nc.tensor.matmul(
                        simT_ps[:],
                        lhsT=gt_sb[:, ci * E:(ci + 1) * E],
                        rhs=ctxT_b[:, ci * BL:(ci + 1) * BL],
                        start=(ci == 0),
                        stop=(ci == GCH - 1),
                    )
                sim_ps = psim.tile([BL, E], DT_F32, tag="sim")
                for ci in range(GCH):
                    nc.tensor.matmul(
                        sim_ps[:],
                        lhsT=ctxT_b[:, ci * BL:(ci + 1) * BL],
                        rhs=gt_sb[:, ci * E:(ci + 1) * E],
                        start=(ci == 0),
                        stop=(ci == GCH - 1),
                    )
                sim_f = lp.tile([BL, E], DT_F32, tag="simf")
                nc.scalar.copy(sim_f[:], sim_ps[:])
                nc.scalar.dma_start(o_sim.ap()[:], sim_f[:])
                simT_b = lp.tile([E, BL], DT_BF, tag="simTb")
                nc.vector.tensor_copy(simT_b[:], simT_ps[:])

                nc.vector.memset(mix[:], 0.0)
                nc.vector.tensor_copy(mix[0:E, :], simT_b[:])
                nc.vector.tensor_copy(mix[96:98, :], xo_sb[96:98, :])
                for j in range(8, 17):
                    gate_k(j, GATE_K_ORDER[j])

              if True:
                sigi = lp.tile([BL, H], DT_F32, tag="t0")
                sigf = lp.tile([BL, H], DT_F32, tag="t1")
                tang = lp.tile([BL, H], DT_F32, tag="t2")
                sigo = lp.tile([BL, H], DT_F32, tag="t3")
                gact = [
                    (sigi, 0, AF.Sigmoid), (sigi, 1, AF.Sigmoid),
                    (sigf, 0, AF.Sigmoid), (sigf, 1, AF.Sigmoid),
                    (tang, 0, AF.Tanh), (tang, 1, AF.Tanh),
                    (sigo, 0, AF.Sigmoid), (sigo, 1, AF.Sigmoid),
                ]
                for n in (2, 3, 0, 1, 4, 5, 6, 7):
                    dst, half, fn = gact[n]
                    nc.scalar.activation(
                        dst[:, half * S:(half + 1) * S], gps[n], fn
                    )

                # cell + LayerNorm in two 512-column halves so ACT and DVE
                # pipeline instead of serializing on [8,1024] ops
                cnew = lp.tile([BL, H], DT_F32, tag="cnew")
                hnew = lp.tile([BL, H], DT_F32, tag="hnew")
                hsums, dhs = [], []
                for q in range(2):
                    sl = slice(q * S, (q + 1) * S)
                    ta = lp.tile([BL, S], DT_F32, tag=f"ta{q}", name=f"ta{q}")
                    nc.vector.tensor_mul(ta[:], sigf[:, sl], c0_sb[:, sl])
                    tb = lp.tile([BL, S], DT_F32, tag=f"tb{q}", name=f"tb{q}")
                    nc.vector.tensor_mul(tb[:], sigi[:, sl], tang[:, sl])
                    nc.vector.tensor_add(cnew[:, sl], ta[:], tb[:])
                    nc.scalar.dma_start(o_c.ap()[:, sl], cnew[:, sl])
                    th = lp.tile([BL, S], DT_F32, tag=f"th{q}", name=f"th{q}")
                    nc.scalar.activation(th[:], cnew[:, sl], AF.Tanh)
                    hs = lp.tile([BL, 1], DT_F32, tag=f"hs{q}", name=f"hs{q}")
                    nc.vector.scalar_tensor_tensor(
                        out=hnew[:, sl], in0=sigo[:, sl], scalar=1.0,
                        op0=ALU.mult, in1=th[:], op1=ALU.mult, accum_out=hs[:],
                    )
                    nc.scalar.dma_start(o_h.ap()[:, sl], hnew[:, sl])
                    hsums.append(hs)

                hsum = lp.tile([BL, 1], DT_F32, tag="hsum")
                nc.vector.tensor_add(hsum[:], hsums[0][:], hsums[1][:])
                mu = lp.tile([BL, 1], DT_F32, tag="mu")
                nc.scalar.mul(mu[:], hsum[:], 1.0 / H)
                sqs = []
                for q in range(2):
                    sl = slice(q * S, (q + 1) * S)
                    dq = lp.tile([BL, S], DT_F32, tag=f"d{q}", name=f"d{q}")
                    nc.vector.tensor_scalar(
                        dq[:], hnew[:, sl], mu[:], None, ALU.subtract
                    )
                    dsq = lp.tile([BL, S], DT_F32, tag=f"ta{q}", name=f"dsq{q}")
                    sq = lp.tile([BL, 1], DT_F32, tag=f"sq{q}", name=f"sq{q}")
                    nc.scalar.activation(dsq[:], dq[:], AF.Square, accum_out=sq[:])
                    dhs.append(dq)
                    sqs.append(sq)
                sqsum = lp.tile([BL, 1], DT_F32, tag="sqsum")
                nc.vector.tensor_add(sqsum[:], sqs[0][:], sqs[1][:])
                eps = lp.tile([BL, 1], DT_F32, tag="eps")
                nc.vector.memset(eps[:], 1e-5)
                sigma = lp.tile([BL, 1], DT_F32, tag="sigma")
                nc.scalar.activation(
                    sigma[:], sqsum[:], AF.Sqrt, scale=1.0 / H, bias=eps[:]
                )
                rstd = lp.tile([BL, 1], DT_F32, tag="rstd")
                nc.vector.reciprocal(rstd[:], sigma[:])
                fins = []
                for q in range(2):
                    sl = slice(q * S, (q + 1) * S)
                    y1 = lp.tile([BL, S], DT_F32, tag=f"tb{q}", name=f"y1_{q}")
                    nc.vector.scalar_tensor_tensor(
                        out=y1[:], in0=dhs[q][:], scalar=rstd[:], op0=ALU.mult,
                        in1=lng_sb[:, sl], op1=ALU.mult,
                    )
                    y2 = lp.tile([BL, S], DT_F32, tag=f"th{q}", name=f"y2_{q}")
                    nc.vector.tensor_add(y2[:], y1[:], lnb_sb[:, sl])
                    yj = lp.tile([BL, S], DT_F32, tag=f"ta{q}", name=f"yj{q}")
                    fc = lp.tile([BL, 1], DT_F32, tag=f"fc{q}", name=f"fc{q}")
                    nc.vector.scalar_tensor_tensor(
                        out=yj[:], in0=y2[:], scalar=1.0, op0=ALU.bypass,
                        in1=wfin_sb[:, sl], op1=ALU.mult, accum_out=fc[:],
                    )
                    fins.append(fc)
                fin_c = lp.tile([BL, 1], DT_F32, tag="fin")
                nc.vector.tensor_add(fin_c[:], fins[0][:], fins[1][:])
                fin_o = lp.tile([BL, 1], DT_F32, tag="fino")
                nc.vector.tensor_add(fin_o[:], fin_c[:], bfin_sb[:])
                nc.scalar.dma_start(o_fin.ap()[:], fin_o[:])

    return nc


def _prep(inputs):
    """Host-side shard + layout prep. Returns per-core input dicts."""
    f32 = np.float32
    x = np.asarray(inputs["x"], f32)
    h0 = np.asarray(inputs["h0"], f32)[0]
    c0 = np.asarray(inputs["c0"], f32)[0]
    enc = np.asarray(inputs["encoder_output"], f32)
    W_enc = np.asarray(inputs["W_enc"], f32)
    W_dec = np.asarray(inputs["W_dec"], f32)
    w_val = np.asarray(inputs["w_val"], f32)
    W_gate = np.asarray(inputs["W_gate"], f32)
    events = np.asarray(inputs["events_mat"], f32)
    W_ih = np.asarray(inputs["W_ih"], f32)
    W_hh = np.asarray(inputs["W_hh"], f32)
    b_ih = np.asarray(inputs["b_ih"], f32)
    b_hh = np.asarray(inputs["b_hh"], f32)
    ln_g = np.asarray(inputs["ln_g"], f32)
    ln_b = np.asarray(inputs["ln_b"], f32)
    W_fin = np.asarray(inputs["W_fin"], f32)
    b_fin = np.asarray(inputs["b_fin"], f32)

    G = W_gate @ events                      # [E, H] weights-only fusion
    # wenct: gi-major pack [p, (gi, ci, 128)]; wdect: ci-major [p, (ci, g)]
    WencT = W_enc.T.reshape(GCH, 128, GCH, 128)
    wenct = np.ascontiguousarray(
        WencT.transpose(1, 2, 0, 3).reshape(128, GCH * H)
    ).astype(BF)
    wdect = np.ascontiguousarray(
        W_dec.T.reshape(GCH, 128, H).transpose(1, 0, 2).reshape(128, GCH * H)
    ).astype(BF)
    wvalm = np.ascontiguousarray(w_val.reshape(GCH, 128).T).astype(BF)
    gtm = np.ascontiguousarray(
        G.T.reshape(GCH, 128, E).transpose(1, 0, 2).reshape(128, GCH * E)
    ).astype(BF)
    wpad = np.zeros((KPAD, G4), f32)
    W_ihT = W_ih.T
    wpad[0:H] = W_ihT[0:H]
    wpad[H:H + E] = W_ihT[H:H + E]        # sim rows -> mix partitions 0..9
    wpad[H + 96] = W_ihT[H + E]           # x row    -> mix partition 96
    wpad[H + 97] = b_ih + b_hh            # bias row -> mix partition 97 (ones)
    wpad[9 * 128:] = W_hh.T
    wpad = wpad.astype(BF)

    maps = []
    for k in range(NCORES):
        sl = slice(k * BL, (k + 1) * BL)
        enct = np.ascontiguousarray(
            enc[sl].transpose(2, 0, 1).reshape(H, BL * S)
        ).astype(BF)
        hloc = h0[sl]                          # [BL, H]
        hts = np.ascontiguousarray(
            hloc.T.reshape(GCH, 128, BL).transpose(1, 0, 2).reshape(128, GCH * BL)
        ).astype(BF)
        maps.append({
            "enct": enct,
            "wenct": wenct,
            "wdect": wdect,
            "wval": wvalm,
            "gt": gtm,
            "wpad": wpad,
            "hts": hts,
            "xo": np.ascontiguousarray(
                np.stack([x[sl, 0], np.ones(BL, np.float32)])
            ).astype(BF),
            "c0f": np.ascontiguousarray(c0[sl]),
            "lngb": np.ascontiguousarray(np.broadcast_to(ln_g, (BL, H))),
            "lnbb": np.ascontiguousarray(np.broadcast_to(ln_b, (BL, H))),
            "wfinb": np.ascontiguousarray(np.broadcast_to(W_fin[0], (BL, H))),
            "bfinb": np.full((BL, 1), b_fin[0], f32),
        })
    return maps


def _run(inputs, trace=False, **kw):
    _patch_tile_tail()
    nc = _build()
    _split_excess_waits(nc)
    in_maps = _prep(inputs)
    res = run_bass_kernel_spmd(nc, in_maps, list(range(NCORES)), trace=trace, **kw)
    r = res.results
    fin = np.concatenate([r[k]["fin"] for k in range(NCORES)], axis=0)
    h_new = np.concatenate([r[k]["hnew"] for k in range(NCORES)], axis=0)[None]
    c_new = np.concatenate([r[k]["cnew"] for k in range(NCORES)], axis=0)[None]
    attn = np.concatenate([r[k]["attnw"] for k in range(NCORES)], axis=0)
    sim = np.concatenate([r[k]["simout"] for k in range(NCORES)], axis=0)
    return (fin, (h_new, c_new), attn, sim), res


def kernel(**inputs):
    out, _ = _run(inputs)
    return out


# revision 42
# speedup vs baseline: 1.4288x; 1.0171x over previous
"""Trainium2 Bass kernel for the attention-LSTM decoder step.

Strategy: data-parallel over batch B=64 across 8 NeuronCores (8 batches/core).
All compute in bf16 with fp32 PSUM accumulation. Host side pre-transposes /
pre-casts inputs (layout only; the one weight-fusion is G = W_gate @ events_mat,
a weights-only product).

Per-core pipeline:
  phase A: decT[g,b] = W_dec @ h0^T            (PE, bf16)
  phase B: per batch b:
    projT[g,s] = W_enc @ enc[b]^T  (+decT bias) -> tanh (ACT)
    scores[s]  = w_val^T tanh(projT)            (PE, M=1 accumulation)
    softmax    (DVE/ACT, [1,512] layout)
    attn broadcast to 128 partitions            (PE ones-outer-product)
    ctxT[h,b]  = sum_s attn[s]*encT[h,s]        (DVE fused mult+reduce)
  phase C: sim scores, LSTM gates = [ctx|sim|x|1|h] @ Wpad (PE), LSTM cell,
           LayerNorm, final projection.
"""

import numpy as np
import ml_dtypes

import concourse.bass as bass
import concourse.tile as tile
import concourse.mybir as mybir
from concourse.vector_clock import ScopedClock
from concourse.bass_utils import run_bass_kernel_spmd

BF = ml_dtypes.bfloat16
DT_BF = mybir.dt.bfloat16
DT_F32 = mybir.dt.float32
AF = mybir.ActivationFunctionType
ALU = mybir.AluOpType
AX = mybir.AxisListType

B, S, H, E = 64, 512, 1024, 10
NCORES = 8
BL = B // NCORES          # 8 local batches
GCH = H // 128            # 8 chunks of the hidden dim
KPAD = 17 * 128           # padded LSTM contraction (ctx 1024 | sim+x+1 pad 128 | h 1024)
G4 = 4 * H                # 4096 gate columns

_PATCHED = False


def _patch_tile_tail():
    """walrus on this image rejects >1 sem-wait on the tail Drain; split the
    waits across single-wait SP nops."""
    global _PATCHED
    if _PATCHED:
        return
    _PATCHED = True

    def _patched(self, tick_clock, wait_clock):
        nc = self.nc
        nop = nc.sync.nop(nofuse=True)
        wait_clock.add_sem_waits(
            nop.ins, ScopedClock({None: tick_clock.global_clock})
        )
        si = nop.ins.sync_info
        waits = list(si.on_wait) if si and si.on_wait else []
        if len(waits) > 1:
            si.on_wait = [waits[0]]
            for w in waits[1:]:
                n2 = nc.sync.nop(nofuse=True)
                n2.ins.sync_info = mybir.SyncInfo(on_wait=[w], on_update=[])
        nc.sync.drain()
        nc.all_engine_barrier()
        assert self.sems is not None
        popped = nc._tile_sem_poison_stack.pop()
        assert popped is self._sem_poison
        nc.clear_and_free_semaphores(list(self.sems.allocated().values()))
        nc.all_engine_barrier()

    tile.TileContext._drain_and_barrier = _patched


def _split_excess_waits(nc, limit=1):
    """walrus codegen on this image rejects >2 sem-waits per instruction.
    Move the excess onto injected same-engine nops placed right before the
    offending instruction (engines execute their stream in order, and a DMA
    trigger is dispatched by its issuing engine, so this is equivalent)."""
    k = 0
    for bb in nc.m.functions[0].blocks:
        insts = bb.instructions
        out = []
        for inst in insts:
            si = inst.sync_info
            waits = list(si.on_wait) if si and si.on_wait else []
            if len(waits) > limit:
                extra, keep = waits[:-limit], waits[-limit:]
                si.on_wait = keep
                for j in range(0, len(extra), limit):
                    k += 1
                    out.append(mybir.InstNoOp(
                        name=f"waitsplit-{k}",
                        engine=inst.engine,
                        bass_nofuse=True,
                        sync_info=mybir.SyncInfo(
                            on_wait=extra[j:j + limit], on_update=[]
                        ),
                    ))
            out.append(inst)
        bb.instructions = out


def _build():
    nc = bass.Bass("TRN2", debug=False)

    def din(name, shape, dt=DT_BF):
        return nc.declare_dram_parameter(name, list(shape), dt, isOutput=False)

    def dout(name, shape, dt=DT_F32):
        return nc.declare_dram_parameter(name, list(shape), dt, isOutput=True)

    enct = din("enct", [H, BL * S])
    wenct = din("wenct", [128, GCH * H])
    wdect = din("wdect", [128, GCH * H])
    wval = din("wval", [128, GCH])
    gt = din("gt", [128, GCH * E])
    wpad = din("wpad", [KPAD, G4])
    hts = din("hts", [128, GCH * BL])
    xo = din("xo", [2, BL])
    c0f = din("c0f", [BL, H], DT_F32)
    lngb = din("lngb", [BL, H], DT_F32)
    lnbb = din("lnbb", [BL, H], DT_F32)
    wfinb = din("wfinb", [BL, H], DT_F32)
    bfinb = din("bfinb", [BL, 1], DT_F32)

    o_fin = dout("fin", [BL, 1])
    o_h = dout("hnew", [BL, H])
    o_c = dout("cnew", [BL, H])
    o_attn = dout("attnw", [BL, S])
    o_sim = dout("simout", [BL, E])

    with tile.TileContext(nc) as tc:
        from contextlib import ExitStack

        ctx = ExitStack()
        with ctx:
            cpool = ctx.enter_context(tc.tile_pool(name="consts", bufs=1))
            wenc_p = ctx.enter_context(tc.tile_pool(name="wenc", bufs=1))
            misc_p = ctx.enter_context(tc.tile_pool(name="misc", bufs=1))
            wpad_p = ctx.enter_context(tc.tile_pool(name="wpad", bufs=24))
            enc_p = ctx.enter_context(tc.tile_pool(name="enc", bufs=16))
            tanh_p = ctx.enter_context(tc.tile_pool(name="tanh", bufs=6))
            junk_p = ctx.enter_context(tc.tile_pool(name="junk", bufs=2))
            sm_p = ctx.enter_context(tc.tile_pool(name="sm", bufs=2))

            decT_sb = misc_p.tile([128, GCH * BL], DT_F32, tag="decT")
            mix = misc_p.tile([128, BL], DT_BF, tag="mix")
            ctxT_f = misc_p.tile([128, GCH * BL], DT_F32, tag="ctxTf")
            ctxT_b = misc_p.tile([128, GCH * BL], DT_BF, tag="ctxTb")

            enc_t = {}

            def load_bg(bg):
                for ci in range(GCH):
                    t = enc_p.tile([128, 2 * S], DT_BF, tag="enc",
                                   name=f"enc{bg}_{ci}")
                    nc.sync.dma_start(
                        t[:],
                        enct.ap()[
                            ci * 128:(ci + 1) * 128, bg * 2 * S:(bg + 1) * 2 * S
                        ],
                    )
                    enc_t[(bg, ci)] = t

            def enc_rhs(b, ci):
                bg, half = divmod(b, 2)
                return enc_t[(bg, ci)][:, half * S:(half + 1) * S]

            # ---- SP-ring DMA order: phase-A weights first, then the first
            # two batch-groups of enc, then the stage-1 weights + misc. ----
            hts_sb = cpool.tile([128, GCH * BL], DT_BF, tag="hts")
            nc.sync.dma_start(hts_sb[:], hts.ap()[:])
            with tc.tile_pool(name="wdec", bufs=1) as wdec_p:
                wdec_sb = wdec_p.tile([128, GCH * H], DT_BF, tag="wdec")
                nc.sync.dma_start(
                    wdec_sb[:, 0:2 * H], wdect.ap()[:, 0:2 * H]
                )
                nc.sync.dma_start(
                    wdec_sb[:, 2 * H:4 * H], wdect.ap()[:, 2 * H:4 * H]
                )
                nc.sync.dma_start(
                    wdec_sb[:, 4 * H:GCH * H], wdect.ap()[:, 4 * H:GCH * H]
                )
                # stage-1 weights in two halves (g0-3 first) + first batches
                wenc_sb = wenc_p.tile([128, GCH * H], DT_BF, tag="wenc")
                nc.sync.dma_start(
                    wenc_sb[:, 0:4 * H], wenct.ap()[:, 0:4 * H]
                )
                load_bg(0)
                nc.sync.dma_start(
                    wenc_sb[:, 4 * H:GCH * H], wenct.ap()[:, 4 * H:GCH * H]
                )
                load_bg(1)
                wval_sb = cpool.tile([128, GCH], DT_BF, tag="wval")
                nc.sync.dma_start(wval_sb[:], wval.ap()[:])
                gt_sb = cpool.tile([128, GCH * E], DT_BF, tag="gt")
                nc.sync.dma_start(gt_sb[:], gt.ap()[:])
                c0_sb = cpool.tile([BL, H], DT_F32, tag="c0")
                nc.sync.dma_start(c0_sb[:], c0f.ap()[:])
                lng_sb = cpool.tile([BL, H], DT_F32, tag="lng")
                nc.sync.dma_start(lng_sb[:], lngb.ap()[:])
                lnb_sb = cpool.tile([BL, H], DT_F32, tag="lnb")
                nc.sync.dma_start(lnb_sb[:], lnbb.ap()[:])
                wfin_sb = cpool.tile([BL, H], DT_F32, tag="wfin")
                nc.sync.dma_start(wfin_sb[:], wfinb.ap()[:])
                bfin_sb = cpool.tile([BL, 1], DT_F32, tag="bfin")
                nc.sync.dma_start(bfin_sb[:], bfinb.ap()[:])
                ones_sb = cpool.tile([1, 128], DT_BF, tag="ones")
                nc.vector.memset(ones_sb[:], 1.0)
                xo_sb = cpool.tile([128, BL], DT_BF, tag="xosb")
                nc.sync.dma_start(xo_sb[96:98, :], xo.ap()[:])
                # preload every ACT function table we use so no table-swap
                # stalls land in the serial tail
                warm = cpool.tile([1, 2], DT_F32, tag="warm")
                nc.vector.memset(warm[:], 0.5)
                for fn in (AF.Tanh, AF.Exp, AF.Sigmoid, AF.Square, AF.Sqrt):
                    nc.scalar.activation(warm[:], warm[:], fn)

                # ---- phase A: decT[g, b]; ci-outer so each wdec tile is
                # consumed as soon as its DMA lands ----
                with tc.tile_pool(name="psd", bufs=1, space="PSUM") as psd:
                    dpss = [
                        psd.tile([128, BL], DT_F32, tag=f"dec{gi}",
                                 name=f"dec{gi}")
                        for gi in range(GCH)
                    ]
                    for ci in range(GCH):
                        for gi in range(GCH):
                            nc.tensor.matmul(
                                dpss[gi][:],
                                lhsT=wdec_sb[:, ci * H + gi * 128:
                                             ci * H + (gi + 1) * 128],
                                rhs=hts_sb[:, ci * BL:(ci + 1) * BL],
                                start=(ci == 0),
                                stop=(ci == GCH - 1),
                                skip_group_check=True,
                            )
                    for gi in range(GCH):
                        nc.scalar.copy(
                            decT_sb[:, gi * BL:(gi + 1) * BL], dpss[gi][:]
                        )

            # ---- LSTM weight prefetch on the ACT HW-DGE ring (its own FIFO,
            # so it streams during phase B instead of queueing behind enc).
            # Emission order MUST match the gates k-loop consumption order so
            # the pool cycles without deadlock. ----
            GATE_K_ORDER = [*range(9, 17), *range(8), 8]
            WPAD_PAIRS = [(ci, q) for ci in GATE_K_ORDER for q in range(2)]
            wpad_t = {}

            def load_wpad(pairs):
                for ci, q in pairs:
                    t = wpad_p.tile([128, 2048], DT_BF, tag="wpad",
                                    name=f"wp{ci}_{q}")
                    nc.sync.dma_start(
                        t[:],
                        wpad.ap()[ci * 128:(ci + 1) * 128, q * 2048:(q + 1) * 2048],
                    )
                    wpad_t[(ci, q)] = t

            load_wpad(WPAD_PAIRS[:18])

            def wpad_rhs(ci, n):
                return wpad_t[(ci, n // 4)][:, (n % 4) * 512:(n % 4 + 1) * 512]

            # ---- phase B ----
            psg = ctx.enter_context(tc.tile_pool(name="psg", bufs=1,
                                                 space="PSUM"))
            gbank = [
                psg.tile([128, S], DT_F32, tag=f"gb{m}", name=f"gb{m}")
                for m in range(2)
            ]

            def gate_lhs(ci):
                if ci < 8:
                    return ctxT_b[:, ci * BL:(ci + 1) * BL]
                if ci == 8:
                    return mix[:]
                return hts_sb[:, (ci - 9) * BL:(ci - 8) * BL]

            # gates: k-major, 2 PSUM banks x 4 PE column-groups.
            gps = [gbank[n % 2][32 * (n // 2):32 * (n // 2) + BL, :]
                   for n in range(8)]

            def gate_k(j, ci):
                for n in range(8):
                    nc.tensor.matmul(
                        gps[n],
                        lhsT=gate_lhs(ci),
                        rhs=wpad_rhs(ci, n),
                        start=(j == 0),
                        stop=(j == 16),
                        skip_group_check=True,
                        tile_position=(0, 32 * (n // 2)),
                    )
            with (
                tc.tile_pool(name="psp", bufs=2, space="PSUM") as psp,
                tc.tile_pool(name="pss", bufs=2, space="PSUM") as pss,
                tc.tile_pool(name="psa", bufs=2, space="PSUM") as psa,
            ):
                score_ps = {}
                pend_score = None
                pend_post = None

                def emit_score(b, gi):
                    def go():
                        tt = tanh_tiles[(b, gi)]
                        nc.tensor.matmul(
                            score_ps[b][:],
                            lhsT=wval_sb[:, gi:gi + 1],
                            rhs=tt[:],
                            start=(gi == 0),
                            stop=(gi == GCH - 1),
                            skip_group_check=True,
                        )
                    return go

                def emit_post(b):
                    # softmax + attn output + broadcast + context accumulation
                    def go():
                        sp = score_ps.pop(b)
                        negmx = sm_p.tile([1, 1], DT_F32, tag="negmx")
                        nc.vector.tensor_reduce(
                            negmx[:], sp[:], axis=AX.X, op=ALU.max, negate=True
                        )
                        exp_sb = sm_p.tile([1, S], DT_F32, tag="exp")
                        ssum = sm_p.tile([1, 1], DT_F32, tag="ssum")
                        nc.scalar.activation(
                            exp_sb[:], sp[:], AF.Exp, bias=negmx[:], accum_out=ssum[:]
                        )
                        rinv = sm_p.tile([1, 1], DT_F32, tag="rinv")
                        nc.vector.reciprocal(rinv[:], ssum[:])
                        attn_f = sm_p.tile([1, S], DT_F32, tag="attnf")
                        nc.vector.tensor_scalar(
                            attn_f[:], exp_sb[:], rinv[:], None, ALU.mult
                        )
                        nc.scalar.dma_start(o_attn.ap()[b:b + 1, :], attn_f[:])
                        attn_b = sm_p.tile([1, S], DT_BF, tag="attnb")
                        nc.vector.tensor_copy(attn_b[:], attn_f[:])
                        aps = psa.tile([128, S], DT_F32, tag="attnb")
                        nc.tensor.matmul(
                            aps[:], lhsT=ones_sb[:], rhs=attn_b[:],
                            start=True, stop=True,
                        )
                        for ci in range(GCH):
                            jt = junk_p.tile([128, S], DT_BF, tag="junk")
                            nc.vector.scalar_tensor_tensor(
                                out=jt[:],
                                in0=enc_rhs(b, ci),
                                scalar=1.0,
                                op0=ALU.bypass,
                                in1=aps[:],
                                op1=ALU.mult,
                                accum_out=ctxT_f[:, ci * BL + b:ci * BL + b + 1],
                            )
                    return go

                tanh_tiles = {}
                for b in range(BL):
                    if b % 2 == 0 and 2 <= b <= 4:
                        load_bg(b // 2 + 1)
                    if b == 6:
                        load_wpad(WPAD_PAIRS[18:])
                    score_ps[b] = pss.tile(
                        [1, S], DT_F32, tag="score", name=f"score{b}"
                    )
                    for gi in range(GCH):
                        proj = psp.tile([128, S], DT_F32, tag="proj")
                        for ci in range(GCH):
                            nc.tensor.matmul(
                                proj[:],
                                lhsT=wenc_sb[:, gi * H + ci * 128:
                                             gi * H + (ci + 1) * 128],
                                rhs=enc_rhs(b, ci),
                                start=(ci == 0),
                                stop=(ci == GCH - 1),
                            )
                        tt = tanh_p.tile([128, S], DT_BF, tag="tanh")
                        col = gi * BL + b
                        nc.scalar.activation(
                            tt[:], proj[:], AF.Tanh, bias=decT_sb[:, col:col + 1]
                        )
                        tanh_tiles[(b, gi)] = tt
                        if pend_score is not None:
                            pend_score()
                            pend_score = None
                        if pend_post is not None:
                            pend_post()
                            pend_post = None
                        pend_score = emit_score(b, gi)
                    # end gi loop: defer score(b,7) + post(b) past proj(b+1,g0)
                    pend_post_prev = pend_post
                    assert pend_post_prev is None
                    pend_post = emit_post(b)
                # epilogue: the h-dependent gates chunks fill the PE while
                # the last batch's softmax/broadcast/context run on ACT/DVE
                if pend_score is not None:
                    pend_score()
                for j in range(8):
                    gate_k(j, GATE_K_ORDER[j])
                if pend_post is not None:
                    pend_post()

            # ---- phase C ----
            nc.vector.tensor_copy(ctxT_b[:], ctxT_f[:])

            with tc.tile_pool(name="lstm", bufs=1) as lp:
              # keep the Sqrt ACT table resident for the LayerNorm tail
              # (phase-B Exp/Tanh traffic evicts it)
              nc.scalar.activation(warm[:], warm[:], AF.Sqrt)
              with tc.tile_pool(name="psim", bufs=1, space="PSUM") as psim:
                simT_ps = psim.tile([E, BL], DT_F32, tag="simT")
                for ci in range(GCH):
                    nc.tensor.matmul(
                        simT_ps[:],
                        lhsT=gt_sb[:, ci * E:(ci + 1) * E],
                        rhs=ctxT_b[:, ci * BL:(ci + 1) * BL],
                        start=(ci == 0),
                        stop=(ci == GCH - 1),
                    )
                sim_ps = psim.tile([BL, E], DT_F32, tag="sim")
                for ci in range(GCH):
                    nc.tensor.matmul(
                        sim_ps[:],
                        lhsT=ctxT_b[:, ci * BL:(ci + 1) * BL],
                        rhs=gt_sb[:, ci * E:(ci + 1) * E],
                        start=(ci == 0),
                        stop=(ci == GCH - 1),
                    )
                sim_f = lp.tile([BL, E], DT_F32, tag="simf")
                nc.scalar.copy(sim_f[:], sim_ps[:])
                nc.scalar.dma_start(o_sim.ap()[:], sim_f[:])
                simT_b = lp.tile([E, BL], DT_BF, tag="simTb")
                nc.vector.tensor_copy(simT_b[:], simT_ps[:])

                nc.vector.memset(mix[:], 0.0)
                nc.vector.tensor_copy(mix[0:E, :], simT_b[:])
                nc.vector.tensor_copy(mix[96:98, :], xo_sb[96:98, :])
                for j in range(8, 17):
                    gate_k(j, GATE_K_ORDER[j])

              if True:
                sigi = lp.tile([BL, H], DT_F32, tag="t0")
                sigf = lp.tile([BL, H], DT_F32, tag="t1")
                tang = lp.tile([BL, H], DT_F32, tag="t2")
                sigo = lp.tile([BL, H], DT_F32, tag="t3")
                gact = [
                    (sigi, 0, AF.Sigmoid), (sigi, 1, AF.Sigmoid),
                    (sigf, 0, AF.Sigmoid), (sigf, 1, AF.Sigmoid),
                    (tang, 0, AF.Tanh), (tang, 1, AF.Tanh),
                    (sigo, 0, AF.Sigmoid), (sigo, 1, AF.Sigmoid),
                ]
                for n in (2, 3, 0, 1, 4, 5, 6, 7):
                    dst, half, fn = gact[n]
                    nc.scalar.activation(
                        dst[:, half * S:(half + 1) * S], gps[n], fn
                    )

                ta = lp.tile([BL, H], DT_F32, tag="t4")
                nc.vector.tensor_mul(ta[:], sigf[:], c0_sb[:])
                tb = lp.tile([BL, H], DT_F32, tag="t5")
                nc.vector.tensor_mul(tb[:], sigi[:], tang[:])
                cnew = lp.tile([BL, H], DT_F32, tag="t6")
                nc.vector.tensor_add(cnew[:], ta[:], tb[:])
                nc.scalar.dma_start(o_c.ap()[:], cnew[:])
                tanhc = lp.tile([BL, H], DT_F32, tag="t4")
                nc.scalar.activation(tanhc[:], cnew[:], AF.Tanh)
                hnew = lp.tile([BL, H], DT_F32, tag="t5")
                hsum = lp.tile([BL, 1], DT_F32, tag="hsum")
                nc.vector.scalar_tensor_tensor(
                    out=hnew[:], in0=sigo[:], scalar=1.0, op0=ALU.mult,
                    in1=tanhc[:], op1=ALU.mult, accum_out=hsum[:],
                )
                nc.scalar.dma_start(o_h.ap()[:], hnew[:])

                mu = lp.tile([BL, 1], DT_F32, tag="mu")
                nc.scalar.mul(mu[:], hsum[:], 1.0 / H)
                d = lp.tile([BL, H], DT_F32, tag="t0")
                nc.vector.tensor_scalar(d[:], hnew[:], mu[:], None, ALU.subtract)
                dsq = lp.tile([BL, H], DT_F32, tag="t1")
                sqsum = lp.tile([BL, 1], DT_F32, tag="sqsum")
                nc.scalar.activation(dsq[:], d[:], AF.Square, accum_out=sqsum[:])
                eps = lp.tile([BL, 1], DT_F32, tag="eps")
                nc.vector.memset(eps[:], 1e-5)
                sigma = lp.tile([BL, 1], DT_F32, tag="sigma")
                nc.scalar.activation(
                    sigma[:], sqsum[:], AF.Sqrt, scale=1.0 / H, bias=eps[:]
                )
                rstd = lp.tile([BL, 1], DT_F32, tag="rstd")
                nc.vector.reciprocal(rstd[:], sigma[:])
                y1 = lp.tile([BL, H], DT_F32, tag="t3")
                nc.vector.scalar_tensor_tensor(
                    out=y1[:], in0=d[:], scalar=rstd[:], op0=ALU.mult,
                    in1=lng_sb[:], op1=ALU.mult,
                )
                y2 = lp.tile([BL, H], DT_F32, tag="t6")
                nc.vector.tensor_add(y2[:], y1[:], lnb_sb[:])
                yj = lp.tile([BL, H], DT_F32, tag="t4")
                fin_c = lp.tile([BL, 1], DT_F32, tag="fin")
                nc.vector.scalar_tensor_tensor(
                    out=yj[:], in0=y2[:], scalar=1.0, op0=ALU.bypass,
                    in1=wfin_sb[:], op1=ALU.mult, accum_out=fin_c[:],
                )
                fin_o = lp.tile([BL, 1], DT_F32, tag="fino")
                nc.vector.tensor_add(fin_o[:], fin_c[:], bfin_sb[:])
                nc.scalar.dma_start(o_fin.ap()[:], fin_o[:])

    return nc.declare_dram_parameter(name, list(shape), dt, isOutput=False)

    def dout(name, shape, dt=DT_F32):
        return nc.declare_dram_parameter(name, list(shape), dt, isOutput=True)

    enct = din("enct", [H, BL * S])
    wenct = din("wenct", [128, GCH * H])
    wdect = din("wdect", [128, GCH * H])
    wval = din("wval", [128, GCH])
    gt = din("gt", [128, GCH * E])
    wpad = din("wpad", [KPAD, G4])
    hts = din("hts", [128, GCH * BL])
    xo = din("xo", [2, BL])
    c0f = din("c0f", [BL, H], DT_F32)
    lngb = din("lngb", [BL, H], DT_F32)
    lnbb = din("lnbb", [BL, H], DT_F32)
    wfinb = din("wfinb", [BL, H], DT_F32)
    bfinb = din("bfinb", [BL, 1], DT_F32)

    o_fin = dout("fin", [BL, 1])
    o_h = dout("hnew", [BL, H])
    o_c = dout("cnew", [BL, H])
    o_attn = dout("attnw", [BL, S])
    o_sim = dout("simout", [BL, E])

    with tile.TileContext(nc) as tc:
        from contextlib import ExitStack

        ctx = ExitStack()
        with ctx:
            cpool = ctx.enter_context(tc.tile_pool(name="consts", bufs=1))
            wenc_p = ctx.enter_context(tc.tile_pool(name="wenc", bufs=1))
            misc_p = ctx.enter_context(tc.tile_pool(name="misc", bufs=1))
            wpad_p = ctx.enter_context(tc.tile_pool(name="wpad", bufs=24))
            enc_p = ctx.enter_context(tc.tile_pool(name="enc", bufs=16))
            tanh_p = ctx.enter_context(tc.tile_pool(name="tanh", bufs=6))
            junk_p = ctx.enter_context(tc.tile_pool(name="junk", bufs=2))
            sm_p = ctx.enter_context(tc.tile_pool(name="sm", bufs=2))

            decT_sb = misc_p.tile([128, GCH * BL], DT_F32, tag="decT")
            mix = misc_p.tile([128, BL], DT_BF, tag="mix")
            ctxT_f = misc_p.tile([128, GCH * BL], DT_F32, tag="ctxTf")
            ctxT_b = misc_p.tile([128, GCH * BL], DT_BF, tag="ctxTb")

            enc_t = {}

            def load_bg(bg):
                for ci in range(GCH):
                    t = enc_p.tile([128, 2 * S], DT_BF, tag="enc",
                                   name=f"enc{bg}_{ci}")
                    nc.sync.dma_start(
                        t[:],
                        enct.ap()[
                            ci * 128:(ci + 1) * 128, bg * 2 * S:(bg + 1) * 2 * S
                        ],
                    )
                    enc_t[(bg, ci)] = t

            def enc_rhs(b, ci):
                bg, half = divmod(b, 2)
                return enc_t[(bg, ci)][:, half * S:(half + 1) * S]

            # ---- SP-ring DMA order: phase-A weights first, then the first
            # two batch-groups of enc, then the stage-1 weights + misc. ----
            hts_sb = cpool.tile([128, GCH * BL], DT_BF, tag="hts")
            nc.sync.dma_start(hts_sb[:], hts.ap()[:])
            with tc.tile_pool(name="wdec", bufs=1) as wdec_p:
                wdec_sb = wdec_p.tile([128, GCH * H], DT_BF, tag="wdec")
                nc.sync.dma_start(
                    wdec_sb[:, 0:2 * H], wdect.ap()[:, 0:2 * H]
                )
                nc.sync.dma_start(
                    wdec_sb[:, 2 * H:4 * H], wdect.ap()[:, 2 * H:4 * H]
                )
                nc.sync.dma_start(
                    wdec_sb[:, 4 * H:GCH * H], wdect.ap()[:, 4 * H:GCH * H]
                )
                # stage-1 weights in two halves (g0-3 first) + first batches
                wenc_sb = wenc_p.tile([128, GCH * H], DT_BF, tag="wenc")
                nc.sync.dma_start(
                    wenc_sb[:, 0:4 * H], wenct.ap()[:, 0:4 * H]
                )
                load_bg(0)
                nc.sync.dma_start(
                    wenc_sb[:, 4 * H:GCH * H], wenct.ap()[:, 4 * H:GCH * H]
                )
                load_bg(1)
                wval_sb = cpool.tile([128, GCH], DT_BF, tag="wval")
                nc.sync.dma_start(wval_sb[:], wval.ap()[:])
                gt_sb = cpool.tile([128, GCH * E], DT_BF, tag="gt")
                nc.sync.dma_start(gt_sb[:], gt.ap()[:])
                c0_sb = cpool.tile([BL, H], DT_F32, tag="c0")
                nc.sync.dma_start(c0_sb[:], c0f.ap()[:])
                lng_sb = cpool.tile([BL, H], DT_F32, tag="lng")
                nc.sync.dma_start(lng_sb[:], lngb.ap()[:])
                lnb_sb = cpool.tile([BL, H], DT_F32, tag="lnb")
                nc.sync.dma_start(lnb_sb[:], lnbb.ap()[:])
                wfin_sb = cpool.tile([BL, H], DT_F32, tag="wfin")
                nc.sync.dma_start(wfin_sb[:], wfinb.ap()[:])
                bfin_sb = cpool.tile([BL, 1], DT_F32, tag="bfin")
                nc.sync.dma_start(bfin_sb[:], bfinb.ap()[:])
                ones_sb = cpool.tile([1, 128], DT_BF, tag="ones")
                nc.vector.memset(ones_sb[:], 1.0)
                xo_sb = cpool.tile([128, BL], DT_BF, tag="xosb")
                nc.sync.dma_start(xo_sb[96:98, :], xo.ap()[:])
                # preload every ACT function table we use so no table-swap
                # stalls land in the serial tail
                warm = cpool.tile([1, 2], DT_F32, tag="warm")
                nc.vector.memset(warm[:], 0.5)
                for fn in (AF.Tanh, AF.Exp, AF.Sigmoid, AF.Square, AF.Sqrt):
                    nc.scalar.activation(warm[:], warm[:], fn)

                # ---- phase A: decT[g, b]; ci-outer so each wdec tile is
                # consumed as soon as its DMA lands ----
                with tc.tile_pool(name="psd", bufs=1, space="PSUM") as psd:
                    dpss = [
                        psd.tile([128, BL], DT_F32, tag=f"dec{gi}",
                                 name=f"dec{gi}")
                        for gi in range(GCH)
                    ]
                    for ci in range(GCH):
                        for gi in range(GCH):
                            nc.tensor.matmul(
                                dpss[gi][:],
                                lhsT=wdec_sb[:, ci * H + gi * 128:
                                             ci * H + (gi + 1) * 128],
                                rhs=hts_sb[:, ci * BL:(ci + 1) * BL],
                                start=(ci == 0),
                                stop=(ci == GCH - 1),
                                skip_group_check=True,
                            )
                    for gi in range(GCH):
                        nc.scalar.copy(
                            decT_sb[:, gi * BL:(gi + 1) * BL], dpss[gi][:]
                        )

            # ---- LSTM weight prefetch on the ACT HW-DGE ring (its own FIFO,
            # so it streams during phase B instead of queueing behind enc).
            # Emission order MUST match the gates k-loop consumption order so
            # the pool cycles without deadlock. ----
            GATE_K_ORDER = [*range(9, 17), *range(8), 8]
            WPAD_PAIRS = [(ci, q) for ci in GATE_K_ORDER for q in range(2)]
            wpad_t = {}

            def load_wpad(pairs):
                for ci, q in pairs:
                    t = wpad_p.tile([128, 2048], DT_BF, tag="wpad",
                                    name=f"wp{ci}_{q}")
                    nc.sync.dma_start(
                        t[:],
                        wpad.ap()[ci * 128:(ci + 1) * 128, q * 2048:(q + 1) * 2048],
                    )
                    wpad_t[(ci, q)] = t

            load_wpad(WPAD_PAIRS[:18])

            def wpad_rhs(ci, n):
                return wpad_t[(ci, n // 4)][:, (n % 4) * 512:(n % 4 + 1) * 512]

            # ---- phase B ----
            psg = ctx.enter_context(tc.tile_pool(name="psg", bufs=1,
                                                 space="PSUM"))
            gbank = [
                psg.tile([128, S], DT_F32, tag=f"gb{m}", name=f"gb{m}")
                for m in range(2)
            ]

            def gate_lhs(ci):
                if ci < 8:
                    return ctxT_b[:, ci * BL:(ci + 1) * BL]
                if ci == 8:
                    return mix[:]
                return hts_sb[:, (ci - 9) * BL:(ci - 8) * BL]

            # gates: k-major, 2 PSUM banks x 4 PE column-groups.
            gps = [gbank[n % 2][32 * (n // 2):32 * (n // 2) + BL, :]
                   for n in range(8)]

            def gate_k(j, ci):
                for n in range(8):
                    nc.tensor.matmul(
                        gps[n],
                        lhsT=gate_lhs(ci),
                        rhs=wpad_rhs(ci, n),
                        start=(j == 0),
                        stop=(j == 16),
                        skip_group_check=True,
                        tile_position=(0, 32 * (n // 2)),
                    )
            with (
                tc.tile_pool(name="psp", bufs=2, space="PSUM") as psp,
                tc.tile_pool(name="pss", bufs=2, space="PSUM") as pss,
                tc.tile_pool(name="psa", bufs=2, space="PSUM") as psa,
            ):
                score_ps = {}
                pend_score = None
                pend_post = None

                def emit_score(b, gi):
                    def go():
                        tt = tanh_tiles[(b, gi)]
                        nc.tensor.matmul(
                            score_ps[b][:],
                            lhsT=wval_sb[:, gi:gi + 1],
                            rhs=tt[:],
                            start=(gi == 0),
                            stop=(gi == GCH - 1),
                            skip_group_check=True,
                        )
                    return go

                def emit_post(b):
                    # softmax + attn output + broadcast + context accumulation
                    def go():
                        sp = score_ps.pop(b)
                        negmx = sm_p.tile([1, 1], DT_F32, tag="negmx")
                        nc.vector.tensor_reduce(
                            negmx[:], sp[:], axis=AX.X, op=ALU.max, negate=True
                        )
                        exp_sb = sm_p.tile([1, S], DT_F32, tag="exp")
                        ssum = sm_p.tile([1, 1], DT_F32, tag="ssum")
                        nc.scalar.activation(
                            exp_sb[:], sp[:], AF.Exp, bias=negmx[:], accum_out=ssum[:]
                        )
                        rinv = sm_p.tile([1, 1], DT_F32, tag="rinv")
                        nc.vector.reciprocal(rinv[:], ssum[:])
                        attn_f = sm_p.tile([1, S], DT_F32, tag="attnf")
                        nc.vector.tensor_scalar(
                            attn_f[:], exp_sb[:], rinv[:], None, ALU.mult
                        )
                        nc.scalar.dma_start(o_attn.ap()[b:b + 1, :], attn_f[:])
                        attn_b = sm_p.tile([1, S], DT_BF, tag="attnb")
                        nc.vector.tensor_copy(attn_b[:], attn_f[:])
                        aps = psa.tile([128, S], DT_F32, tag="attnb")
                        nc.tensor.matmul(
                            aps[:], lhsT=ones_sb[:], rhs=attn_b[:],
                            start=True, stop=True,
                        )
                        for ci in range(GCH):
                            jt = junk_p.tile([128, S], DT_BF, tag="junk")
                            nc.vector.scalar_tensor_tensor(
                                out=jt[:],
                                in0=enc_rhs(b, ci),
                                scalar=1.0,
                                op0=ALU.bypass,
                                in1=aps[:],
                                op1=ALU.mult,
                                accum_out=ctxT_f[:, ci * BL + b:ci * BL + b + 1],
                            )
                    return go

                tanh_tiles = {}
                for b in range(BL):
                    if b % 2 == 0 and 2 <= b <= 4:
                        load_bg(b // 2 + 1)
                    if b == 6:
                        load_wpad(WPAD_PAIRS[18:])
                    score_ps[b] = pss.tile(
                        [1, S], DT_F32, tag="score", name=f"score{b}"
                    )
                    for gi in range(GCH):
                        proj = psp.tile([128, S], DT_F32, tag="proj")
                        for ci in range(GCH):
                            nc.tensor.matmul(
                                proj[:],
                                lhsT=wenc_sb[:, gi * H + ci * 128:
                                             gi * H + (ci + 1) * 128],
                                rhs=enc_rhs(b, ci),
                                start=(ci == 0),
                                stop=(ci == GCH - 1),
                            )
                        tt = tanh_p.tile([128, S], DT_BF, tag="tanh")
                        col = gi * BL + b
                        nc.scalar.activation(
                            tt[:], proj[:], AF.Tanh, bias=decT_sb[:, col:col + 1]
                        )
                        tanh_tiles[(b, gi)] = tt
                        if pend_score is not None:
                            pend_score()
                            pend_score = None
                        if pend_post is not None:
                            pend_post()
                            pend_post = None
                        pend_score = emit_score(b, gi)
                    # end gi loop: defer score(b,7) + post(b) past proj(b+1,g0)
                    pend_post_prev = pend_post
                    assert pend_post_prev is None
                    pend_post = emit_post(b)
                # epilogue: the h-dependent gates chunks fill the PE while
                # the last batch's softmax/broadcast/context run on ACT/DVE
                if pend_score is not None:
                    pend_score()
                for j in range(8):
                    gate_k(j, GATE_K_ORDER[j])
                if pend_post is not None:
                    pend_post()

            # ---- phase C ----
            nc.vector.tensor_copy(ctxT_b[:], ctxT_f[:])

            with tc.tile_pool(name="lstm", bufs=1) as lp:
              # keep the Sqrt ACT table resident for the LayerNorm tail
              # (phase-B Exp/Tanh traffic evicts it)
              nc.scalar.activation(warm[:], warm[:], AF.Sqrt)
              with tc.tile_pool(name="psim", bufs=1, space="PSUM") as psim:
                simT_ps = psim.tile([E, BL], DT_F32, tag="simT")
                for ci in range(GCH):
                    nc.tensor.matmul(
                        simT_ps[:],
                        lhsT=gt_sb[:, ci * E:(ci + 1) * E],
                        rhs=ctxT_b[:, ci * BL:(ci + 1) * BL],
                        start=(ci == 0),
                        stop=(ci == GCH - 1),
                    )
                sim_ps = psim.tile([BL, E], DT_F32, tag="sim")
                for ci in range(GCH):
                    nc.tensor.matmul(
                        sim_ps[:],
                        lhsT=ctxT_b[:, ci * BL:(ci + 1) * BL],
                        rhs=gt_sb[:, ci * E:(ci + 1) * E],
                        start=(ci == 0),
                        stop=(ci == GCH - 1),
                    )
                sim_f = lp.tile([BL, E], DT_F32, tag="simf")
                nc.scalar.copy(sim_f[:], sim_ps[:])
                nc.scalar.dma_start(o_sim.ap()[:], sim_f[:])
                simT_b = lp.tile([E, BL], DT_BF, tag="simTb")
                nc.vector.tensor_copy(simT_b[:], simT_ps[:])

                nc.vector.memset(mix[:], 0.0)
                nc.vector.tensor_copy(mix[0:E, :], simT_b[:])
                nc.vector.tensor_copy(mix[96:98, :], xo_sb[96:98, :])
                for j in range(8, 17):
                    gate_k(j, GATE_K_ORDER[j])

              if True:
                sigi = lp.tile([BL, H], DT_F32, tag="t0")
                sigf = lp.tile([BL, H], DT_F32, tag="t1")
                tang = lp.tile([BL, H], DT_F32, tag="t2")
                sigo = lp.tile([BL, H], DT_F32, tag="t3")
                gact = [
                    (sigi, 0, AF.Sigmoid), (sigi, 1, AF.Sigmoid),
                    (sigf, 0, AF.Sigmoid), (sigf, 1, AF.Sigmoid),
                    (tang, 0, AF.Tanh), (tang, 1, AF.Tanh),
                    (sigo, 0, AF.Sigmoid), (sigo, 1, AF.Sigmoid),
                ]
                for n in (2, 3, 0, 1, 4, 5, 6, 7):
                    dst, half, fn = gact[n]
                    nc.scalar.activation(
                        dst[:, half * S:(half + 1) * S], gps[n], fn
                    )

                # cell + LayerNorm in two 512-column halves so ACT and DVE
                # pipeline instead of serializing on [8,1024] ops
                cnew = lp.tile([BL, H], DT_F32, tag="cnew")
                hnew = lp.tile([BL, H], DT_F32, tag="hnew")
                hsums, dhs = [], []
                for q in range(2):
                    sl = slice(q * S, (q + 1) * S)
                    ta = lp.tile([BL, S], DT_F32, tag=f"ta{q}", name=f"ta{q}")
                    nc.vector.tensor_mul(ta[:], sigf[:, sl], c0_sb[:, sl])
                    tb = lp.tile([BL, S], DT_F32, tag=f"tb{q}", name=f"tb{q}")
                    nc.vector.tensor_mul(tb[:], sigi[:, sl], tang[:, sl])
                    nc.vector.tensor_add(cnew[:, sl], ta[:], tb[:])
                    nc.scalar.dma_start(o_c.ap()[:, sl], cnew[:, sl])
                    th = lp.tile([BL, S], DT_F32, tag=f"th{q}", name=f"th{q}")
                    nc.scalar.activation(th[:], cnew[:, sl], AF.Tanh)
                    hs = lp.tile([BL, 1], DT_F32, tag=f"hs{q}", name=f"hs{q}")
                    nc.vector.scalar_tensor_tensor(
                        out=hnew[:, sl], in0=sigo[:, sl], scalar=1.0,
                        op0=ALU.mult, in1=th[:], op1=ALU.mult, accum_out=hs[:],
                    )
                    nc.scalar.dma_start(o_h.ap()[:, sl], hnew[:, sl])
                    hsums.append(hs)

                hsum = lp.tile([BL, 1], DT_F32, tag="hsum")
                nc.vector.tensor_add(hsum[:], hsums[0][:], hsums[1][:])
                mu = lp.tile([BL, 1], DT_F32, tag="mu")
                nc.scalar.mul(mu[:], hsum[:], 1.0 / H)
                sqs = []
                for q in range(2):
                    sl = slice(q * S, (q + 1) * S)
                    dq = lp.tile([BL, S], DT_F32, tag=f"d{q}", name=f"d{q}")
                    nc.vector.tensor_scalar(
                        dq[:], hnew[:, sl], mu[:], None, ALU.subtract
                    )
                    dsq = lp.tile([BL, S], DT_F32, tag=f"ta{q}", name=f"dsq{q}")
                    sq = lp.tile([BL, 1], DT_F32, tag=f"sq{q}", name=f"sq{q}")
                    nc.scalar.activation(dsq[:], dq[:], AF.Square, accum_out=sq[:])
                    dhs.append(dq)
                    sqs.append(sq)
                sqsum = lp.tile([BL, 1], DT_F32, tag="sqsum")
                nc.vector.tensor_add(sqsum[:], sqs[0][:], sqs[1][:])
                eps = lp.tile([BL, 1], DT_F32, tag="eps")
                nc.vector.memset(eps[:], 1e-5)
                sigma = lp.tile([BL, 1], DT_F32, tag="sigma")
                nc.scalar.activation(
                    sigma[:], sqsum[:], AF.Sqrt, scale=1.0 / H, bias=eps[:]
                )
                rstd = lp.tile([BL, 1], DT_F32, tag="rstd")
                nc.vector.reciprocal(rstd[:], sigma[:])
                fins = []
                for q in range(2):
                    sl = slice(q * S, (q + 1) * S)
                    y1 = lp.tile([BL, S], DT_F32, tag=f"tb{q}", name=f"y1_{q}")
                    nc.vector.scalar_tensor_tensor(
                        out=y1[:], in0=dhs[q][:], scalar=rstd[:], op0=ALU.mult,
                        in1=lng_sb[:, sl], op1=ALU.mult,
                    )
                    y2 = lp.tile([BL, S], DT_F32, tag=f"th{q}", name=f"y2_{q}")
                    nc.vector.tensor_add(y2[:], y1[:], lnb_sb[:, sl])
                    yj = lp.tile([BL, S], DT_F32, tag=f"ta{q}", name=f"yj{q}")
                    fc = lp.tile([BL, 1], DT_F32, tag=f"fc{q}", name=f"fc{q}")
                    nc.vector.scalar_tensor_tensor(
                        out=yj[:], in0=y2[:], scalar=1.0, op0=ALU.bypass,
                        in1=wfin_sb[:, sl], op1=ALU.mult, accum_out=fc[:],
                    )
                    fins.append(fc)
                fin_c = lp.tile([BL, 1], DT_F32, tag="fin")
                nc.vector.tensor_add(fin_c[:], fins[0][:], fins[1][:])
                fin_o = lp.tile([BL, 1], DT_F32, tag="fino")
                nc.vector.tensor_add(fin_o[:], fin_c[:], bfin_sb[:])
                nc.scalar.dma_start(o_fin.ap()[:], fin_o[:])

    return nc


def _prep(inputs):
    """Host-side shard + layout prep. Returns per-core input dicts."""
    f32 = np.float32
    x = np.asarray(inputs["x"], f32)
    h0 = np.asarray(inputs["h0"], f32)[0]
    c0 = np.asarray(inputs["c0"], f32)[0]
    enc = np.asarray(inputs["encoder_output"], f32)
    W_enc = np.asarray(inputs["W_enc"], f32)
    W_dec = np.asarray(inputs["W_dec"], f32)
    w_val = np.asarray(inputs["w_val"], f32)
    W_gate = np.asarray(inputs["W_gate"], f32)
    events = np.asarray(inputs["events_mat"], f32)
    W_ih = np.asarray(inputs["W_ih"], f32)
    W_hh = np.asarray(inputs["W_hh"], f32)
    b_ih = np.asarray(inputs["b_ih"], f32)
    b_hh = np.asarray(inputs["b_hh"], f32)
    ln_g = np.asarray(inputs["ln_g"], f32)
    ln_b = np.asarray(inputs["ln_b"], f32)
    W_fin = np.asarray(inputs["W_fin"], f32)
    b_fin = np.asarray(inputs["b_fin"], f32)

    G = W_gate @ events                      # [E, H] weights-only fusion
    # wenct: gi-major pack [p, (gi, ci, 128)]; wdect: ci-major [p, (ci, g)]
    WencT = W_enc.T.reshape(GCH, 128, GCH, 128)
    wenct = np.ascontiguousarray(
        WencT.transpose(1, 2, 0, 3).reshape(128, GCH * H)
    ).astype(BF)
    wdect = np.ascontiguousarray(
        W_dec.T.reshape(GCH, 128, H).transpose(1, 0, 2).reshape(128, GCH * H)
    ).astype(BF)
    wvalm = np.ascontiguousarray(w_val.reshape(GCH, 128).T).astype(BF)
    gtm = np.ascontiguousarray(
        G.T.reshape(GCH, 128, E).transpose(1, 0, 2).reshape(128, GCH * E)
    ).astype(BF)
    wpad = np.zeros((KPAD, G4), f32)
    W_ihT = W_ih.T
    wpad[0:H] = W_ihT[0:H]
    wpad[H:H + E] = W_ihT[H:H + E]        # sim rows -> mix partitions 0..9
    wpad[H + 96] = W_ihT[H + E]           # x row    -> mix partition 96
    wpad[H + 97] = b_ih + b_hh            # bias row -> mix partition 97 (ones)
    wpad[9 * 128:] = W_hh.T
    wpad = wpad.astype(BF)

    maps = []
    for k in range(NCORES):
        sl = slice(k * BL, (k + 1) * BL)
        enct = np.ascontiguousarray(
            enc[sl].transpose(2, 0, 1).reshape(H, BL * S)
        ).astype(BF)
        hloc = h0[sl]                          # [BL, H]
        hts = np.ascontiguousarray(
            hloc.T.reshape(GCH, 128, BL).transpose(1, 0, 2).reshape(128, GCH * BL)
        ).astype(BF)
        maps.append({
            "enct": enct,
            "wenct": wenct,
            "wdect": wdect,
            "wval": wvalm,
            "gt": gtm,
            "wpad": wpad,
            "hts": hts,
            "xo": np.ascontiguousarray(
                np.stack([x[sl, 0], np.ones(BL, np.float32)])
            ).astype(BF),
            "c0f": np.ascontiguousarray(c0[sl]),
            "lngb": np.ascontiguousarray(np.broadcast_to(ln_g, (BL, H))),
            "lnbb": np.ascontiguousarray(np.broadcast_to(ln_b, (BL, H))),
            "wfinb": np.ascontiguousarray(np.broadcast_to(W_fin[0], (BL, H))),
            "bfinb": np.full((BL, 1), b_fin[0], f32),
        })
    return maps


def _run(inputs, trace=False, **kw):
    _patch_tile_tail()
    nc = _build()
    _split_excess_waits(nc)
    in_maps = _prep(inputs)
    res = run_bass_kernel_spmd(nc, in_maps, list(range(NCORES)), trace=trace, **kw)
    r = res.results
    fin = np.concatenate([r[k]["fin"] for k in range(NCORES)], axis=0)
    h_new = np.concatenate([r[k]["hnew"] for k in range(NCORES)], axis=0)[None]
    c_new = np.concatenate([r[k]["cnew"] for k in range(NCORES)], axis=0)[None]
    attn = np.concatenate([r[k]["attnw"] for k in range(NCORES)], axis=0)
    sim = np.concatenate([r[k]["simout"] for k in range(NCORES)], axis=0)
    return (fin, (h_new, c_new), attn, sim), res


def kernel(**inputs):
    out, _ = _run(inputs)
    return out
